# revision 1
# baseline (speedup 1.0000x reference)
"""Self-contained Trainium2 Bass kernel for GQA MultiHeadAttention with RoPE.

Problem: B=2, S=2048, D=1024, H=16 Q heads, KVH=4 KV heads, head_dim=64,
causal additive mask, f32.

Sharding: tensor-parallel over heads (TP=4: 4 Q heads + 1 KV head per shard)
x data-parallel over batch (DP=2) = 8 NeuronCores. Wo is sharded on its
input dim; the host sums the 4 partial outputs per batch element.
"""

import os
import sys

for _p in ("/opt/trn_rl_repo", "/root/.axon_site/_ro/trn_rl_repo"):
    if os.path.isdir(_p) and _p not in sys.path:
        sys.path.insert(0, _p)

import numpy as np
import ml_dtypes

import concourse.bacc as bacc
import concourse.bass as bass
import concourse.tile as tile
from concourse import mybir
from concourse.bass_utils import run_bass_kernel_spmd

F32 = mybir.dt.float32
F32R = mybir.dt.float32r
BF16 = mybir.dt.bfloat16
AF = mybir.ActivationFunctionType

H, KVH, HD = 16, 4, 64
B, S, D = 2, 2048, 1024
TP = 4                      # head-parallel ways
SCALE = HD ** -0.5
NEG = -1e9
NT = S // 128               # 16 kv tiles
NQB = S // 512              # 4 q blocks

PHASES = "D"                # profiling knob: stop after phase A/B/C/D


def _patch_act_tables():
    """Make Exp/Ln resolve only to natural_log_exp_and_others so the
    act-table-load pass emits one load instead of thrashing between the
    exp-only and ln-only sets."""
    from concourse.hw_specs import get_activation_tables
    t = get_activation_tables("gen3")
    for name, fns in t.items():
        if name != "natural_log_exp_and_others":
            fns.discard(AF.Exp)
            fns.discard(AF.Ln)


def _build_nc(causal: bool):
    _patch_act_tables()
    nc = bacc.Bacc()

    hT = nc.declare_dram_parameter("hT", [D, S], BF16, isOutput=False)
    cs64 = nc.declare_dram_parameter("cs64", [64, S], F32, isOutput=False)
    sn64 = nc.declare_dram_parameter("sn64", [64, S], F32, isOutput=False)
    wq = nc.declare_dram_parameter("wq", [D, 256], BF16, isOutput=False)
    wkv = nc.declare_dram_parameter("wkv", [D, 128], BF16, isOutput=False)
    wo = nc.declare_dram_parameter("wo", [256, D], F32R, isOutput=False)
    psigT = nc.declare_dram_parameter("psigT", [128, 128], F32R, isOutput=False)
    ident = nc.declare_dram_parameter("ident", [128, 128], F32R, isOutput=False)
    m01 = nc.declare_dram_parameter("m01", [128, 128], F32, isOutput=False)
    ones16 = nc.declare_dram_parameter("ones16", [128, 16], F32R, isOutput=False)
    outp = nc.declare_dram_parameter("out", [S, D], F32, isOutput=True)
    rscratch = nc.dram_tensor("rscratch", [4 * NQB, 512], F32)
    rscratch2 = nc.dram_tensor("rscratch2", [NQB * 2048], F32)

    with tile.TileContext(nc) as tc:
        with tc.tile_pool(name="hold", bufs=1) as hp:
            # weights first (small), then hidden in column halves so the
            # first projection chunk can start at ~half the load time
            wq_sb = hp.tile([128, 8, 256], BF16, name="wq_sb", tag="wq_sb")
            nc.sync.dma_start(out=wq_sb, in_=wq.rearrange("(c p) n -> p c n", p=128))
            wkv_sb = hp.tile([128, 8, 128], BF16, name="wkv_sb", tag="wkv_sb")
            nc.sync.dma_start(out=wkv_sb, in_=wkv.rearrange("(c p) n -> p c n", p=128))
            cosf_sb = hp.tile([128, S], F32, name="cosf_sb", tag="cosf_sb")
            sinf_sb = hp.tile([128, S], F32, name="sinf_sb", tag="sinf_sb")
            nc.sync.dma_start(out=cosf_sb[0:64, :], in_=cs64[:, :])
            nc.sync.dma_start(out=cosf_sb[64:128, :], in_=cs64[:, :])
            nc.sync.dma_start(out=sinf_sb[0:64, :], in_=sn64[:, :])
            nc.sync.dma_start(out=sinf_sb[64:128, :], in_=sn64[:, :])
            psig_sb = hp.tile([128, 128], F32R, name="psig_sb", tag="psig_sb")
            nc.sync.dma_start(out=psig_sb, in_=psigT[:, :])
            id_sb = hp.tile([128, 128], F32R, name="id_sb", tag="id_sb")
            nc.sync.dma_start(out=id_sb, in_=ident[:, :])
            m01_sb = hp.tile([128, 128], F32, name="m01_sb", tag="m01_sb")
            nc.sync.dma_start(out=m01_sb, in_=m01[:, :])
            wo_sb = hp.tile([128, 2, D], F32R, name="wo_sb", tag="wo_sb")
            nc.sync.dma_start(out=wo_sb,
                              in_=wo.rearrange("(c p) n -> p c n", p=128))

            ht_sb = [hp.tile([128, S], BF16, name=f"ht{c}", tag=f"ht{c}")
                     for c in range(8)]
            for half in range(2):
                hsl = slice(1024 * half, 1024 * half + 1024)
                for c in range(8):
                    nc.sync.dma_start(out=ht_sb[c][:, hsl],
                                      in_=hT[c * 128:(c + 1) * 128, hsl])

            qTs = [hp.tile([128, S], F32R, name=f"qT{p}", tag=f"qT{p}")
                   for p in range(2)]
            kT = hp.tile([128, S], F32R, name="kTt", tag="kTt")
            vsm = hp.tile([128, NT, 65], F32R, name="vsm", tag="vsm")
            ctxTs = [[hp.tile([128, 512], F32R, name=f"ctxT{c}_{q}",
                              tag=f"ctxT{c}_{q}") for q in range(NQB)]
                     for c in range(2)]

            # ones column (64) of vsm for the softmax denominator row
            nc.sync.dma_start(out=vsm[:, :, 64:65],
                              in_=ones16.rearrange("p (n o) -> p n o", o=1))

            # ---- Phases A-D share one PSUM budget via tag-sharing ----
            with tc.tile_pool(name="psS", bufs=1, space="PSUM") as psS, \
                 tc.tile_pool(name="psC", bufs=1, space="PSUM") as psC, \
                 tc.tile_pool(name="etp", bufs=1) as etp, \
                 tc.tile_pool(name="sbA", bufs=3) as sbA, \
                 tc.tile_pool(name="sbC", bufs=1) as sbC:

                # ---------------- Phase A: projections + rope ----------------
                def emit_q(pp):
                    for sc in range(4):
                        csl = slice(512 * sc, 512 * sc + 512)
                        ps_q = psS.tile([128, 1024], F32, name="ps_q",
                                        tag="ps_s", bufs=2)[:, 0:512]
                        for dc in range(8):
                            nc.tensor.matmul(
                                ps_q,
                                wq_sb[:, dc, 128 * pp:128 * pp + 128],
                                ht_sb[dc][:, csl],
                                start=(dc == 0), stop=(dc == 7))
                        qraw = sbA.tile([128, 512], F32R, name="qraw", tag="qraw")
                        nc.scalar.copy(qraw, ps_q)
                        ps_rot = psS.tile([128, 512], F32, name="ps_rot",
                                          tag="ps_d", bufs=2)
                        nc.tensor.matmul(ps_rot, psig_sb.bitcast(F32R),
                                         qraw.bitcast(F32R), start=True, stop=True)
                        dst = qTs[pp][:, csl]
                        nc.vector.tensor_mul(dst, qraw.bitcast(F32),
                                             cosf_sb[:, csl])
                        rtmp = sbA.tile([128, 512], F32, name="rtmp", tag="rtmp")
                        nc.vector.tensor_mul(rtmp, ps_rot, sinf_sb[:, csl])
                        nc.vector.tensor_add(dst, dst.bitcast(F32), rtmp)

                def emit_kv():
                    # K/V: kvT = [Wk|Wv].T @ h.T -> K rows 0:64, V rows 64:128
                    for sc in range(4):
                        csl = slice(512 * sc, 512 * sc + 512)
                        ps_kv = psS.tile([128, 1024], F32, name="ps_kv",
                                         tag="ps_s", bufs=2)[:, 0:512]
                        for dc in range(8):
                            nc.tensor.matmul(
                                ps_kv,
                                wkv_sb[:, dc, :],
                                ht_sb[dc][:, csl],
                                start=(dc == 0), stop=(dc == 7))
                        kvraw = sbA.tile([128, 512], F32R, name="kvraw",
                                         tag="kvraw")
                        nc.scalar.copy(kvraw, ps_kv)
                        # rope on K rows
                        ps_krot = psS.tile([128, 512], F32, name="ps_krot",
                                           tag="ps_d", bufs=2)[0:64, :]
                        nc.tensor.matmul(ps_krot,
                                         psig_sb[0:64, 0:64].bitcast(F32R),
                                         kvraw[0:64, :].bitcast(F32R),
                                         start=True, stop=True)
                        kdst = kT[0:64, csl]
                        nc.vector.tensor_mul(kdst, kvraw[0:64, :].bitcast(F32),
                                             cosf_sb[0:64, csl])
                        ktmp = sbA.tile([64, 512], F32, name="ktmp", tag="ktmp")
                        nc.vector.tensor_mul(ktmp, ps_krot, sinf_sb[0:64, csl])
                        nc.vector.tensor_add(kdst, kdst.bitcast(F32), ktmp)
                        # V: transpose each 128-seq tile into vsm (seq-major)
                        for tt in range(4):
                            ti = 4 * sc + tt
                            ps_v = psC.tile([128, 512], F32, name="ps_v",
                                            tag="ps_ctx", bufs=2)[:, 0:64]
                            nc.tensor.matmul(
                                ps_v.bitcast(F32R),
                                kvraw[64:128, 128 * tt:128 * tt + 128].bitcast(F32R),
                                id_sb[64:128, 0:64].bitcast(F32R),
                                start=True, stop=True, is_transpose=True)
                            nc.vector.tensor_copy(vsm[:, ti, 0:64], ps_v)
                    # duplicate roped K to partitions 64:128 so odd heads can
                    # use base-64 aligned operands (engines cannot cross
                    # partitions; DMA can)
                    nc.sync.dma_start(out=kT[64:128, :], in_=kT[0:64, :])

                def emit_phase_d(dq):
                    for qt in range(4 * dq, 4 * dq + 4):
                        for nb in range(2):
                            ps_o = psS.tile([128, 512], F32, name="ps_o",
                                            tag="ps_d", bufs=2)
                            for c in range(2):
                                ct = ctxTs[c][qt // 4]
                                col = 128 * (qt % 4)
                                nc.tensor.matmul(
                                    ps_o,
                                    ct[:, col:col + 128].bitcast(F32R),
                                    wo_sb[:, c, 512 * nb:512 * nb + 512].bitcast(F32R),
                                    start=(c == 0), stop=(c == 1))
                            ost = sbC.tile([128, 512], F32, name="ost",
                                           tag="ost", bufs=4)
                            if nb == 0:
                                nc.vector.tensor_copy(ost, ps_o)
                            else:
                                nc.scalar.copy(ost, ps_o)
                            nc.sync.dma_start(
                                out=outp[128 * qt:128 * qt + 128,
                                         512 * nb:512 * nb + 512],
                                in_=ost)

                def emit_bc(qb, sp, last=False):
                    # attention + normalization for one (q block, slot pair)
                    ctxu = sbC.tile([65, 1024], F32, name="ctxu", tag="ctxu",
                                    bufs=3)
                    for hh in range(2):
                        h = 2 * sp + hh
                        off = 64 * (h % 2)
                        pp = h // 2
                        ps_ctx = psC.tile([128, 512], F32, name="ps_ctx",
                                          tag="ps_ctx", bufs=2)
                        nki = (4 * qb + 4) if causal else NT
                        nfull = (4 * qb) if causal else NT
                        # software-pipelined tile units: emit the NEXT unit's
                        # scores matmuls before this unit's ctx matmuls so PE
                        # never waits on the exp
                        units = []

                        def mk_pair(kp, _off=off, _pp=pp, _qb=qb, _nki=nki,
                                    _ps_ctx=ps_ctx):
                            box = {}

                            def s():
                                ps_s = psS.tile([128, 1024], F32, name="ps_s",
                                                tag="ps_s", bufs=2)
                                for jj in range(2):
                                    ki = kp + jj
                                    nc.tensor.matmul(
                                        ps_s[:, 512 * jj:512 * jj + 512],
                                        kT[_off:_off + 64,
                                           128 * ki:128 * ki + 128].bitcast(F32R),
                                        qTs[_pp][_off:_off + 64,
                                            512 * _qb:512 * _qb + 512].bitcast(F32R),
                                        start=True, stop=True)
                                box["ps"] = ps_s

                            def ec():
                                et = etp.tile([128, 1024], F32R, name="et",
                                              tag="et", bufs=4)
                                nc.scalar.activation(et, box["ps"], AF.Exp,
                                                     scale=SCALE)
                                for jj in range(2):
                                    ki = kp + jj
                                    nc.tensor.matmul(
                                        _ps_ctx[0:65, :],
                                        vsm[:, ki, 0:65].bitcast(F32R),
                                        et[:, 512 * jj:512 * jj + 512].bitcast(F32R),
                                        start=(ki == 0), stop=(ki == _nki - 1))
                            return (s, ec)

                        def mk_diag(j, _off=off, _pp=pp, _qb=qb, _nki=nki,
                                    _ps_ctx=ps_ctx):
                            box = {}
                            ki = 4 * _qb + j
                            soff, span = 128 * j, 512 - 128 * j

                            def s():
                                ps_d = psS.tile([128, 512], F32, name="ps_d",
                                                tag="ps_d", bufs=2)
                                nc.tensor.matmul(
                                    ps_d[:, :span],
                                    kT[_off:_off + 64,
                                       128 * ki:128 * ki + 128].bitcast(F32R),
                                    qTs[_pp][_off:_off + 64,
                                        512 * _qb + soff:512 * (_qb + 1)].bitcast(F32R),
                                    start=True, stop=True)
                                box["ps"] = ps_d

                            def ec():
                                etd = etp.tile([128, 512], F32R, name="etd",
                                               tag="etd", bufs=4)
                                nc.scalar.activation(etd[:, :span],
                                                     box["ps"][:, :span],
                                                     AF.Exp, scale=SCALE)
                                ceng = nc.gpsimd if j % 2 == 0 else nc.vector
                                ceng.tensor_mul(etd[:, :128],
                                                etd[:, :128].bitcast(F32),
                                                m01_sb)
                                nc.tensor.matmul(
                                    _ps_ctx[0:65, soff:512],
                                    vsm[:, ki, 0:65].bitcast(F32R),
                                    etd[:, :span].bitcast(F32R),
                                    start=(ki == 0), stop=(ki == _nki - 1))
                            return (s, ec)

                        for kp in range(0, nfull, 2):
                            units.append(mk_pair(kp))
                        if causal:
                            for j in range(4):
                                units.append(mk_diag(j))
                        if units:
                            units[0][0]()
                        for i in range(len(units)):
                            if i + 1 < len(units):
                                units[i + 1][0]()
                            units[i][1]()
                        # evict unnormalized ctx + rowsum, freeing psum
                        nc.vector.tensor_copy(
                            ctxu[0:65, 512 * hh:512 * hh + 512],
                            ps_ctx[0:65, :])
                    if PHASES == "B":
                        return
                    # ---- phase C: batched reciprocal of the 2 rowsum rows ----
                    sbase = 2048 * qb + 1024 * sp
                    if last:
                        # tail fast path: ln/exp directly on the (idle) ACT at
                        # 1-partition width, skipping the [128,8] reshape hops
                        nc.scalar.activation(ctxu[64:65, :], ctxu[64:65, :],
                                             AF.Ln)
                        nc.scalar.activation(ctxu[64:65, :], ctxu[64:65, :],
                                             AF.Exp, scale=-1.0)
                        s_ap = rscratch2[sbase:sbase + 1024]
                        nc.sync.dma_start(
                            out=bass.AP(tensor=s_ap.tensor, offset=s_ap.offset,
                                        ap=[[1, 1], [1, 1024]]),
                            in_=ctxu[64:65, :])
                    else:
                        for hh in range(2):
                            slot = 4 * qb + 2 * sp + hh
                            nc.sync.dma_start(
                                out=rscratch[slot, :],
                                in_=ctxu[64:65, 512 * hh:512 * hh + 512])
                        rs = sbC.tile([128, 8], F32, name="rs", tag="rs", bufs=2)
                        g_ap = rscratch[4 * qb + 2 * sp]
                        nc.sync.dma_start(
                            out=rs, in_=bass.AP(tensor=g_ap.tensor,
                                                offset=g_ap.offset,
                                                ap=[[8, 128], [1, 8]]))
                        nc.scalar.activation(rs, rs, AF.Ln)
                        nc.scalar.activation(rs, rs, AF.Exp, scale=-1.0)
                        s_ap = rscratch2[sbase:sbase + 1024]
                        nc.sync.dma_start(
                            out=bass.AP(tensor=s_ap.tensor, offset=s_ap.offset,
                                        ap=[[8, 128], [1, 8]]), in_=rs)
                    if sp == 1:
                        # move the cross-partition hop off the critical path:
                        # copy UNNORMALIZED ctx to base 64 now (depends only
                        # on the psum evict), normalize in place once the
                        # reciprocal arrives
                        for hh in range(2):
                            nc.sync.dma_start(
                                out=ctxTs[hh][qb][64:128, :],
                                in_=ctxu[0:64,
                                         512 * hh:512 * hh + 512].bitcast(F32R))
                    for hh in range(2):
                        ct = ctxTs[hh][qb]
                        if sp == 0:
                            rb = sbC.tile([64, 512], F32, name="rb", tag="rb",
                                          bufs=4)
                            r_ap = rscratch2[sbase + 512 * hh:
                                             sbase + 512 * (hh + 1)]
                            nc.gpsimd.dma_start(
                                out=rb, in_=bass.AP(tensor=r_ap.tensor,
                                                    offset=r_ap.offset,
                                                    ap=[[0, 64], [1, 512]]))
                            nc.vector.tensor_mul(
                                ct[0:64, :],
                                ctxu[0:64, 512 * hh:512 * hh + 512], rb)
                        else:
                            rb = sbC.tile([128, 512], F32, name="rbw",
                                          tag="rbw", bufs=4)
                            r_ap = rscratch2[sbase + 512 * hh:
                                             sbase + 512 * (hh + 1)]
                            nc.gpsimd.dma_start(
                                out=rb, in_=bass.AP(tensor=r_ap.tensor,
                                                    offset=r_ap.offset,
                                                    ap=[[0, 128], [1, 512]]))
                            nc.vector.tensor_mul(
                                ct[64:128, :], ct[64:128, :].bitcast(F32),
                                rb[64:128, :])

                # ---- global emission order: overlap phase A with qb=0 ----
                emit_kv()
                emit_q(0)
                if PHASES == "A":
                    emit_q(1)
                else:
                    emit_bc(0, 0)
                    emit_q(1)
                    emit_bc(0, 1)
                    for qb in range(1, NQB):
                        emit_bc(qb, 0, last=(qb == NQB - 1))
                        emit_bc(qb, 1, last=(qb == NQB - 1))
                        if PHASES == "D":
                            emit_phase_d(qb - 1)
                    if PHASES == "D":
                        emit_phase_d(NQB - 1)

    nc.compile()
    return nc


_NC_CACHE = {}


def _get_nc(causal: bool):
    if causal not in _NC_CACHE:
        _NC_CACHE[causal] = _build_nc(causal)
    return _NC_CACHE[causal]


def _host_consts():
    p = np.zeros((128, 128), np.float32)
    idx = np.arange(0, 128, 2)
    p[idx, idx + 1] = -1.0
    p[idx + 1, idx] = 1.0
    psigT = np.ascontiguousarray(p.T)
    ident = np.eye(128, dtype=np.float32)
    ident[64:128, 0:64] = np.eye(64, dtype=np.float32)
    m01 = (np.arange(128)[None, :] >= np.arange(128)[:, None]).astype(np.float32)
    return psigT, ident, m01


def _numpy_reference(hidden_states, cos, sin, attention_mask, Wq, Wk, Wv, Wo):
    """Generic-mask fallback, pure numpy port of the reference."""
    GROUPS = H // KVH

    def rope(x, c, s):
        c = c[:, None, :, :]
        s = s[:, None, :, :]
        x1, x2 = x[..., ::2], x[..., 1::2]
        xr = np.stack([x1 * c - x2 * s, x1 * s + x2 * c], axis=-1)
        return xr.reshape(x.shape)

    b, sq, d = hidden_states.shape
    q = (hidden_states @ Wq).reshape(b, sq, H, HD).transpose(0, 2, 1, 3)
    k = (hidden_states @ Wk).reshape(b, sq, KVH, HD).transpose(0, 2, 1, 3)
    v = (hidden_states @ Wv).reshape(b, sq, KVH, HD).transpose(0, 2, 1, 3)
    q = rope(q, cos, sin)
    k = rope(k, cos, sin)
    k = np.repeat(k, GROUPS, axis=1)
    v = np.repeat(v, GROUPS, axis=1)
    out = np.zeros((b, sq, d), np.float32)
    for bi in range(b):
        for hi in range(H):
            sc = (q[bi, hi] @ k[bi, hi].T) * SCALE + attention_mask[0, 0]
            sc = sc - sc.max(axis=-1, keepdims=True)
            e = np.exp(sc)
            pr = e / e.sum(axis=-1, keepdims=True)
            ctx = pr @ v[bi, hi]
            out[bi] += ctx @ Wo[hi * HD:(hi + 1) * HD]
    return out


def kernel(**inputs) -> np.ndarray:
    hs = np.asarray(inputs["hidden_states"], np.float32)
    cos = np.asarray(inputs["cos"], np.float32)
    sin = np.asarray(inputs["sin"], np.float32)
    mask = np.asarray(inputs["attention_mask"], np.float32)
    Wq = np.asarray(inputs["Wq"], np.float32)
    Wk = np.asarray(inputs["Wk"], np.float32)
    Wv = np.asarray(inputs["Wv"], np.float32)
    Wo = np.asarray(inputs["Wo"], np.float32)

    m = mask.reshape(S, S)
    tril = np.tril(np.ones((S, S), dtype=bool))
    causal_ref = np.where(tril, np.float32(0.0), np.float32(NEG))
    if np.array_equal(m, causal_ref):
        causal = True
    elif not m.any():
        causal = False
    else:
        return _numpy_reference(hs, cos, sin, mask, Wq, Wk, Wv, Wo)

    nc = _get_nc(causal)
    psigT, ident, m01 = _host_consts()
    chan_half = (np.arange(64) // 2)

    in_maps = []
    for core in range(8):
        b, t = core // TP, core % TP
        hT = np.ascontiguousarray(hs[b].T).astype(ml_dtypes.bfloat16)
        cs64v = np.ascontiguousarray(cos[b].T[chan_half, :])
        sn64v = np.ascontiguousarray(sin[b].T[chan_half, :])
        wq_s = np.ascontiguousarray(
            Wq[:, t * 256:(t + 1) * 256]).astype(ml_dtypes.bfloat16)
        wkv_s = np.ascontiguousarray(
            np.concatenate([Wk[:, t * 64:(t + 1) * 64],
                            Wv[:, t * 64:(t + 1) * 64]],
                           axis=1)).astype(ml_dtypes.bfloat16)
        wo_s = Wo[t * 256:(t + 1) * 256]
        # ctxT channel order per chunk: c0 = [h0|h2], c1 = [h1|h3]
        wo_p = np.ascontiguousarray(
            np.concatenate([wo_s[0:64], wo_s[128:192],
                            wo_s[64:128], wo_s[192:256]], axis=0))
        in_maps.append({
            "hT": hT, "cs64": cs64v, "sn64": sn64v,
            "wq": wq_s, "wkv": wkv_s, "wo": wo_p,
            "psigT": psigT, "ident": ident, "m01": m01,
            "ones16": np.ones((128, 16), np.float32),
        })

    res = run_bass_kernel_spmd(nc, in_maps, core_ids=list(range(8)))
    out = np.zeros((B, S, D), np.float32)
    for core in range(8):
        out[core // TP] += res.results[core]["out"]
    return out



# revision 43
# speedup vs baseline: 1.3008x; 1.3008x over previous
"""Self-contained Trainium2 Bass kernel for GQA MultiHeadAttention with RoPE.

Problem: B=2, S=2048, D=1024, H=16 Q heads, KVH=4 KV heads, head_dim=64,
causal additive mask, f32.

Sharding: tensor-parallel over heads (TP=4: 4 Q heads + 1 KV head per shard)
x data-parallel over batch (DP=2) = 8 NeuronCores. Wo is sharded on its
input dim; the host sums the 4 partial outputs per batch element.

Design notes (all bf16 datapath, f32 psum):
 - scores kept kv-major ([kv, q] psum tiles) so exp feeds strictly from PE;
   causal diagonal handled by a -1e9 mask ADDED via a PT @ I matmul into the
   same psum accumulation group (no post-exp mask multiplies).
 - ctx computed q-major: stationary = exp'd scores chunk [kv,128q], moving =
   V||ones [kv,65]  ->  psum [128q, 65].  The softmax denominator lands on
   column 64, per-partition, so normalize = reciprocal + tensor_scalar_mul,
   no DRAM round trip.
 - ctx transposed back to ch-major for the output projection with the
   DMA xbar transpose (SBUF->SBUF, bf16).
 - Activation engine runs exps only; evictions go to DVE/ACT split; GPSIMD
   cannot touch PSUM.
"""

import os
import sys

for _p in ("/opt/trn_rl_repo", "/root/.axon_site/_ro/trn_rl_repo"):
    if os.path.isdir(_p) and _p not in sys.path:
        sys.path.insert(0, _p)

import numpy as np
import ml_dtypes

import concourse.bacc as bacc
import concourse.bass as bass
import concourse.tile as tile
from concourse import mybir
from concourse.bass_utils import run_bass_kernel_spmd

F32 = mybir.dt.float32
BF16 = mybir.dt.bfloat16
AF = mybir.ActivationFunctionType
BF = ml_dtypes.bfloat16

H, KVH, HD = 16, 4, 64
B, S, D = 2, 2048, 1024
TP = 4                      # head-parallel ways
SCALE = HD ** -0.5
NEG = -1e9
NT = S // 128               # 16 kv tiles
NQB = S // 512              # 4 q blocks


def _patch_act_tables():
    """Make Exp resolve only to natural_log_exp_and_others so the act-table
    pass emits a single table load."""
    from concourse.hw_specs import get_activation_tables
    t = get_activation_tables("gen3")
    for name, fns in t.items():
        if name != "natural_log_exp_and_others":
            fns.discard(AF.Exp)
            fns.discard(AF.Ln)


def _build_nc(causal: bool):
    _patch_act_tables()
    nc = bacc.Bacc()

    hT = nc.declare_dram_parameter("hT", [128, 8, S], BF16, isOutput=False)
    csd = nc.declare_dram_parameter("csd", [128, S], BF16, isOutput=False)
    snd = nc.declare_dram_parameter("snd", [128, S], BF16, isOutput=False)
    wq = nc.declare_dram_parameter("wq", [128, 8, 256], BF16, isOutput=False)
    wkv = nc.declare_dram_parameter("wkv", [128, 8, 128], BF16, isOutput=False)
    wo = nc.declare_dram_parameter("wo", [128, 2, D], BF16, isOutput=False)
    psigT = nc.declare_dram_parameter("psigT", [128, 128], BF16, isOutput=False)
    ptneg = nc.declare_dram_parameter("ptneg", [128, 128], BF16, isOutput=False)
    ident = nc.declare_dram_parameter("ident", [128, 128], BF16, isOutput=False)
    m384 = nc.declare_dram_parameter("m384", [128, 384], BF16, isOutput=False)
    outp = nc.declare_dram_parameter("out", [S, D], BF16, isOutput=True)

    with tile.TileContext(nc) as tc:
        with tc.tile_pool(name="hold", bufs=1) as hp:
            # ---- constants / weights (two DMA queues) ----
            # single sync queue, strict priority order for the first-exp path
            wkv_sb = hp.tile([128, 8, 128], BF16, name="wkv_sb", tag="wkv_sb")
            nc.sync.dma_start(out=wkv_sb, in_=wkv[:, :, :])
            ht_sb = hp.tile([128, 8, S], BF16, name="ht_sb", tag="ht_sb")
            # first column chunk in two halves so kv matmuls start early
            nc.sync.dma_start(out=ht_sb[:, 0:4, 0:512], in_=hT[:, 0:4, 0:512])
            nc.sync.dma_start(out=ht_sb[:, 4:8, 0:512], in_=hT[:, 4:8, 0:512])
            cos_sb = hp.tile([128, S], BF16, name="cos_sb", tag="cos_sb")
            sin_sb = hp.tile([128, S], BF16, name="sin_sb", tag="sin_sb")
            nc.sync.dma_start(out=cos_sb[:, 0:512], in_=csd[:, 0:512])
            nc.sync.dma_start(out=sin_sb[:, 0:512], in_=snd[:, 0:512])
            psig_sb = hp.tile([128, 128], BF16, name="psig_sb", tag="psig_sb")
            nc.sync.dma_start(out=psig_sb, in_=psigT[:, :])
            pt_sb = hp.tile([128, 128], BF16, name="pt_sb", tag="pt_sb")
            nc.sync.dma_start(out=pt_sb, in_=ptneg[:, :])
            id_sb = hp.tile([128, 128], BF16, name="id_sb", tag="id_sb")
            nc.sync.dma_start(out=id_sb, in_=ident[:, :])
            m384_sb = hp.tile([128, 384], BF16, name="m384_sb", tag="m384_sb")
            nc.sync.dma_start(out=m384_sb, in_=m384[:, :])
            wq_sb = hp.tile([128, 8, 256], BF16, name="wq_sb", tag="wq_sb")
            nc.sync.dma_start(out=wq_sb, in_=wq[:, :, :])
            nc.sync.dma_start(out=ht_sb[:, :, 512:1024], in_=hT[:, :, 512:1024])
            nc.sync.dma_start(out=cos_sb[:, 512:S], in_=csd[:, 512:S])
            nc.sync.dma_start(out=sin_sb[:, 512:S], in_=snd[:, 512:S])
            for sc in range(2, 4):
                csl = slice(512 * sc, 512 * sc + 512)
                nc.sync.dma_start(out=ht_sb[:, :, csl], in_=hT[:, :, csl])
            wo_sb = hp.tile([128, 2, D], BF16, name="wo_sb", tag="wo_sb")
            nc.sync.dma_start(out=wo_sb, in_=wo[:, :, :])

            qTs = [hp.tile([128, S], BF16, name=f"qT{p}", tag=f"qT{p}")
                   for p in range(2)]
            kT = hp.tile([128, S], BF16, name="kTt", tag="kTt")
            vsm = hp.tile([128, NT, 65], BF16, name="vsm", tag="vsm")
            nc.gpsimd.memset(vsm[:, :, 64:65], 1.0)
            zer_sb = hp.tile([128, 512], BF16, name="zer_sb", tag="zer_sb")
            nc.gpsimd.memset(zer_sb, 0.0)

            with tc.tile_pool(name="psS", bufs=1, space="PSUM") as psS, \
                 tc.tile_pool(name="psD", bufs=1, space="PSUM") as psD, \
                 tc.tile_pool(name="psQ", bufs=1, space="PSUM") as psQ, \
                 tc.tile_pool(name="etp", bufs=1) as etp, \
                 tc.tile_pool(name="sbA", bufs=1) as sbA, \
                 tc.tile_pool(name="sbC", bufs=1) as sbC:

                # per-qb rotating ctx tiles (q-major and transposed ch-major)
                # A: heads 0,1 (ch 0:128); B: heads 2,3 (ch 128:256)
                ctxq = [sbC.tile([128, 4, 128], BF16, name=f"ctxq{h2}",
                                 tag=f"ctxq{h2}", bufs=2) for h2 in range(2)]
                ctxT = [sbC.tile([128, 4, 128], BF16, name=f"ctxT{h2}",
                                 tag=f"ctxT{h2}", bufs=2) for h2 in range(2)]

                state = {"ctxq": [ctxq[0], ctxq[1]],
                         "ctxT": [ctxT[0], ctxT[1]],
                         "outq": []}

                def flush_outq():
                    for dst, ost in state["outq"]:
                        nc.sync.dma_start(out=dst, in_=ost)
                    state["outq"] = []

                # ---------------- Phase A: projections + rope ----------------
                # Emitted as ~0.5us micro-steps so interleaving into the BC
                # unit stream never starves the exp cadence.
                def a_psum(tag):
                    # prologue can borrow the (idle) score banks
                    if tag == "ps_s":
                        return psS.tile([128, 1024], F32, name="ps_a",
                                        tag="ps_s", bufs=2)[:, 0:512]
                    return psD.tile([128, 512], F32, name="ps_a",
                                    tag="ps_d", bufs=2)

                def steps_kv(sc, tag="ps_d"):
                    csl = slice(512 * sc, 512 * sc + 512)
                    box = {}

                    def proj(d0):
                        if d0 == 0:
                            box["ps"] = a_psum(tag)
                            box["kvraw"] = sbA.tile([128, 512], BF16,
                                                    name="kvraw", tag="kvraw",
                                                    bufs=2)
                        for dc in (d0, d0 + 1):
                            nc.tensor.matmul(box["ps"], wkv_sb[:, dc, :],
                                             ht_sb[:, dc, csl],
                                             start=(dc == 0), stop=(dc == 7))
                        if d0 == 6:
                            nc.vector.tensor_copy(box["kvraw"], box["ps"])

                    def krot():
                        ps_kr = a_psum(tag)[0:64, :]
                        kvraw = box["kvraw"]
                        nc.tensor.matmul(ps_kr, psig_sb[0:64, 0:64],
                                         kvraw[0:64, :], start=True,
                                         stop=True)
                        kdst = kT[0:64, csl]
                        nc.vector.tensor_mul(kdst, kvraw[0:64, :],
                                             cos_sb[0:64, csl])
                        ktmp = sbA.tile([64, 512], BF16, name="ktmp",
                                        tag="ktmp", bufs=2)
                        nc.vector.tensor_mul(ktmp, ps_kr, sin_sb[0:64, csl])
                        nc.vector.tensor_add(kdst, kdst, ktmp)

                    def vt(tt):
                        ti = 4 * sc + tt
                        ps_v = a_psum(tag)
                        ps_vb = ps_v.bitcast(BF16)[:, 0:64]
                        nc.tensor.matmul(
                            ps_vb,
                            box["kvraw"][64:128, 128 * tt:128 * tt + 128],
                            id_sb[64:128, 64:128],
                            start=True, stop=True, is_transpose=True)
                        nc.vector.tensor_copy(vsm[:, ti, 0:64], ps_vb)

                    return ([lambda d0=d0: proj(d0) for d0 in (0, 2, 4, 6)]
                            + [krot]
                            + [lambda tt=tt: vt(tt) for tt in range(4)])

                def steps_q(sc, pp, tag="ps_d"):
                    csl = slice(512 * sc, 512 * sc + 512)
                    box = {}

                    def proj(d0):
                        if d0 == 0:
                            if pp == 1:
                                # deferred K duplicate (waits settled by now)
                                nc.sync.dma_start(out=kT[64:128, csl],
                                                  in_=kT[0:64, csl])
                            box["ps"] = a_psum(tag)
                            box["qraw"] = sbA.tile([128, 512], BF16,
                                                   name="qraw", tag="qraw",
                                                   bufs=2)
                        for dc in (d0, d0 + 1):
                            nc.tensor.matmul(
                                box["ps"],
                                wq_sb[:, dc, 128 * pp:128 * pp + 128],
                                ht_sb[:, dc, csl],
                                start=(dc == 0), stop=(dc == 7))
                        if d0 == 6:
                            nc.vector.tensor_copy(box["qraw"], box["ps"])

                    def qrot():
                        qraw = box["qraw"]
                        ps_r = a_psum(tag)
                        nc.tensor.matmul(ps_r, psig_sb, qraw, start=True,
                                         stop=True)
                        dst = qTs[pp][:, csl]
                        nc.vector.tensor_mul(dst, qraw, cos_sb[:, csl])
                        rtmp = sbA.tile([128, 512], BF16, name="rtmp",
                                        tag="rtmp", bufs=2)
                        nc.vector.tensor_mul(rtmp, ps_r, sin_sb[:, csl])
                        nc.vector.tensor_add(dst, dst, rtmp)

                    return ([lambda d0=d0: proj(d0) for d0 in (0, 2, 4, 6)]
                            + [qrot])

                # ---------------- Phase BC: attention ----------------
                def emit_bc(qb, filler):
                    """Attention for q block qb, 4 heads; unit-pipelined.

                    filler: list of closures emitting independent PE work,
                    popped between units to cover exp latency.
                    """
                    qsl = slice(512 * qb, 512 * qb + 512)
                    nfull = 4 * qb if causal else NT
                    pending = []
                    n_units = 4 * ((nfull + 1) // 2 + (2 if causal else 0))
                    bstate = {"left": max(n_units, 1), "carry": 0.0}

                    def boundary(flush=False):
                        # deferred emissions first (their waits are settled),
                        # then evenly-paced independent PE filler work
                        for _ in range(len(pending)):
                            pending.pop(0)()
                        if flush:
                            n = len(filler)
                        else:
                            bstate["carry"] += len(filler) / bstate["left"]
                            n = int(bstate["carry"])
                            bstate["carry"] -= n
                            bstate["left"] = max(bstate["left"] - 1, 1)
                        for _ in range(n):
                            if filler:
                                filler.pop(0)()

                    for h in range(4):
                        off = 64 * (h % 2)
                        pp = h // 2
                        ps_qm = psQ.tile([128, 4, 128], F32, name="ps_qm",
                                         tag="ps_qm", bufs=2)
                        # one accumulation group per psum bank: zero the whole
                        # bank once, then every ctx matmul accumulates.  After
                        # the first per-qc stop the bank flag is cleared, so
                        # later accumulates skip the group check.
                        nc.tensor.matmul(ps_qm[:, :, :].opt(), pt_sb, zer_sb,
                                         start=True, stop=False)
                        hstate = {"stopped": False}

                        def ctx_mm(ki, et_ap, qc):
                            # ctx[qc] += et_chunk.T @ (V||1)
                            last = (4 * qb + qc) if causal else (NT - 1)
                            stop = (ki == last)
                            nc.tensor.matmul(
                                ps_qm[:, qc, 0:65], et_ap, vsm[:, ki, :],
                                start=False, stop=stop,
                                skip_group_check=hstate["stopped"])
                            if stop:
                                hstate["stopped"] = True

                        units = []

                        def mk_pair(kp, _off=off, _pp=pp):
                            box = {}

                            def s():
                                ps_s = psS.tile([128, 1024], F32, name="ps_s",
                                                tag="ps_s", bufs=2)
                                for jj in range(2):
                                    ki = kp + jj
                                    nc.tensor.matmul(
                                        ps_s[:, 512 * jj:512 * jj + 512],
                                        kT[_off:_off + 64,
                                           128 * ki:128 * ki + 128],
                                        qTs[_pp][_off:_off + 64, qsl],
                                        start=True, stop=True)
                                box["ps"] = ps_s

                            def ec():
                                et = etp.tile([128, 1024], BF16, name="et",
                                              tag="et", bufs=4)
                                nc.scalar.activation(et, box["ps"], AF.Exp,
                                                     scale=SCALE)
                                for qc in range(4):
                                    for jj in range(2):
                                        ctx_mm(kp + jj,
                                               et[:, 512 * jj + 128 * qc:
                                                  512 * jj + 128 * qc + 128],
                                               qc)
                            return (s, ec)

                        def mk_diagA(_off=off, _pp=pp, _qb=qb):
                            # kv tiles 4qb+0 (span 512) and 4qb+1 (span 384)
                            box = {}
                            k0 = 4 * _qb

                            def s():
                                ps = psS.tile([128, 1024], F32, name="ps_dA",
                                              tag="ps_s", bufs=2)
                                nc.tensor.matmul(
                                    ps[:, 0:512],
                                    kT[_off:_off + 64,
                                       128 * k0:128 * k0 + 128],
                                    qTs[_pp][_off:_off + 64, qsl],
                                    start=True, stop=False)
                                nc.tensor.matmul(
                                    ps[:, 0:128], pt_sb, id_sb,
                                    start=False, stop=True)
                                nc.tensor.matmul(
                                    ps[:, 512:896],
                                    kT[_off:_off + 64,
                                       128 * k0 + 128:128 * k0 + 256],
                                    qTs[_pp][_off:_off + 64,
                                             512 * _qb + 128:512 * _qb + 512],
                                    start=True, stop=False)
                                nc.tensor.matmul(
                                    ps[:, 512:640], pt_sb, id_sb,
                                    start=False, stop=True)
                                box["ps"] = ps

                            def ec():
                                et = etp.tile([128, 896], BF16, name="etdA",
                                              tag="etdA", bufs=2)
                                nc.scalar.activation(et, box["ps"][:, 0:896],
                                                     AF.Exp, scale=SCALE)
                                for qc in range(4):
                                    ctx_mm(k0, et[:, 128 * qc:128 * qc + 128],
                                           qc)
                                for qc in range(1, 4):
                                    ctx_mm(k0 + 1,
                                           et[:, 512 + 128 * (qc - 1):
                                              512 + 128 * qc], qc)
                            return (s, ec)

                        def mk_diagB(_off=off, _pp=pp, _qb=qb):
                            # kv tiles 4qb+2 (span 256) and 4qb+3 (span 128)
                            box = {}
                            k2 = 4 * _qb + 2

                            def s():
                                ps = psS.tile([128, 1024], F32, name="ps_dB",
                                              tag="ps_s", bufs=2)
                                # one group per bank: the [P|0|P] mask
                                # preload writes (and zeroes) every byte the
                                # exp will read, then scores accumulate
                                nc.tensor.matmul(
                                    ps[:, 0:384], pt_sb, m384_sb,
                                    start=True, stop=False)
                                nc.tensor.matmul(
                                    ps[:, 0:256],
                                    kT[_off:_off + 64,
                                       128 * k2:128 * k2 + 128],
                                    qTs[_pp][_off:_off + 64,
                                             512 * _qb + 256:512 * _qb + 512],
                                    start=False, stop=False)
                                nc.tensor.matmul(
                                    ps[:, 256:384],
                                    kT[_off:_off + 64,
                                       128 * k2 + 128:128 * k2 + 256],
                                    qTs[_pp][_off:_off + 64,
                                             512 * _qb + 384:512 * _qb + 512],
                                    start=False, stop=True)
                                box["ps"] = ps

                            def ec():
                                et = etp.tile([128, 384], BF16, name="etdB",
                                              tag="etdB", bufs=2)
                                nc.scalar.activation(et, box["ps"][:, 0:384],
                                                     AF.Exp, scale=SCALE)
                                for qc in range(2, 4):
                                    ctx_mm(k2, et[:, 128 * (qc - 2):
                                                  128 * (qc - 1)], qc)
                                ctx_mm(k2 + 1, et[:, 256:384], 3)
                            return (s, ec)

                        for kp in range(0, nfull, 2):
                            units.append(mk_pair(kp))
                        if causal:
                            units.append(mk_diagA())
                            units.append(mk_diagB())

                        # normalize + evict q-major ctx (bf16); deferred one
                        # unit so the semaphore waits never clog DVE's queue
                        def normalize(_h=h, _ps=ps_qm, _qb=qb, qcs=(0, 1, 2, 3),
                                      tail_h=False):
                            rcp = sbC.tile([128, len(qcs)], F32, name="rcp",
                                           tag="rcp", bufs=4)
                            nc.vector.reciprocal_approx_fast(
                                rcp, _ps[:, qcs[0]:qcs[-1] + 1, 64:65].opt())
                            dstq = state["ctxq"][_h // 2]
                            dstT = state["ctxT"][_h // 2]
                            col = 64 * (_h % 2)
                            for i, qc in enumerate(qcs):
                                nc.vector.tensor_scalar_mul(
                                    dstq[:, qc, col:col + 64],
                                    _ps[:, qc, 0:64], rcp[:, i:i + 1])
                                if tail_h:
                                    # progressive per-qc transpose right
                                    # behind each normalize chunk, on
                                    # alternating queues
                                    eng = nc.sync if qc % 2 == 0 else nc.scalar
                                    eng.dma_start_transpose(
                                        dstT[:, qc, :], dstq[:, qc, :])
                            if _h % 2 == 1 and not tail_h:
                                def transpose(_h2=_h // 2):
                                    dT = state["ctxT"][_h2]
                                    dq = state["ctxq"][_h2]
                                    nc.sync.dma_start_transpose(
                                        dT[:, :, :], dq[:, :, :])
                                pending.append(transpose)

                        is_tail_h = (causal and qb == NQB - 1 and h == 3)
                        if units:
                            units[0][0]()
                        for i in range(len(units)):
                            if i + 1 < len(units):
                                units[i + 1][0]()
                            boundary()
                            units[i][1]()
                            if is_tail_h and i == len(units) - 2:
                                # qc0/1 groups stopped in diagA: normalize
                                # them while diagB is still in flight
                                pending.append(
                                    lambda: normalize(qcs=(0, 1),
                                                      tail_h=True))

                        if is_tail_h:
                            pending.append(lambda: normalize(qcs=(2, 3),
                                                             tail_h=True))
                        else:
                            pending.append(normalize)
                    while pending:
                        pending.pop(0)()
                    boundary(flush=True)

                # ---------------- Phase D: output projection ----------------
                def emit_d(qb, ctxTA, ctxTB, tail=False):
                    out = []
                    for qt in range(4):
                        for nb in range(2):
                            def f(_qt=qt, _nb=nb, _qb=qb, _A=ctxTA, _B=ctxTB):
                                ps_o = psD.tile([128, 512], F32, name="ps_o",
                                                tag="ps_d", bufs=2)
                                nsl = slice(512 * _nb, 512 * _nb + 512)
                                nc.tensor.matmul(ps_o, _A[:, _qt, :],
                                                 wo_sb[:, 0, nsl],
                                                 start=True, stop=False)
                                nc.tensor.matmul(ps_o, _B[:, _qt, :],
                                                 wo_sb[:, 1, nsl],
                                                 start=False, stop=True)
                                ost = sbC.tile([128, 512], BF16, name="ost",
                                               tag="ost", bufs=16)
                                if tail and _nb == 1:
                                    nc.scalar.copy(ost, ps_o)
                                else:
                                    nc.vector.tensor_copy(ost, ps_o)
                                dst = outp[512 * _qb + 128 * _qt:
                                           512 * _qb + 128 * _qt + 128, nsl]
                                if tail:
                                    nc.sync.dma_start(out=dst, in_=ost)
                                else:
                                    # deferred so its wait is settled by the
                                    # time it hits the SP queue
                                    state["outq"].append((dst, ost))
                            out.append(f)
                    return out

                # ---------------- global schedule ----------------
                def afiller(sc):
                    return (steps_kv(sc) + steps_q(sc, 0) + steps_q(sc, 1))

                def rotate_ctx():
                    state["ctxq"] = [
                        sbC.tile([128, 4, 128], BF16, name=f"ctxq{h2}",
                                 tag=f"ctxq{h2}", bufs=2) for h2 in range(2)]
                    state["ctxT"] = [
                        sbC.tile([128, 4, 128], BF16, name=f"ctxT{h2}",
                                 tag=f"ctxT{h2}", bufs=2) for h2 in range(2)]

                if causal:
                    # wavefront: BC(qb) only needs kv/q chunks <= qb.
                    # Prologue: kv proj -> q proj (borrowing score banks) ->
                    # k rope -> q rope; V transposes land in BC(0)'s filler.
                    kv0 = steps_kv(0)
                    q00 = steps_q(0, 0, tag="ps_s")
                    for st in kv0[0:4]:         # kv proj
                        st()
                    for st in q00[0:4]:         # q proj (ps_s banks)
                        st()
                    kv0[4]()                    # k rope
                    q00[4]()                    # q rope
                    emit_bc(0, kv0[5:9] + steps_q(0, 1) + afiller(1))
                    for qb in range(1, NQB):
                        dA = emit_d(qb - 1, state["ctxT"][0],
                                    state["ctxT"][1])
                        rotate_ctx()
                        fill = (afiller(qb + 1) if qb + 1 < NQB else [])
                        fill += [flush_outq] + dA
                        emit_bc(qb, fill)
                else:
                    # full attention needs all kv before any BC
                    for sc in range(4):
                        for st in steps_kv(sc) + steps_q(sc, 0) \
                                + steps_q(sc, 1):
                            st()
                    emit_bc(0, [])
                    for qb in range(1, NQB):
                        dA = emit_d(qb - 1, state["ctxT"][0],
                                    state["ctxT"][1])
                        rotate_ctx()
                        emit_bc(qb, [flush_outq] + dA)
                for f in emit_d(NQB - 1, state["ctxT"][0], state["ctxT"][1],
                                tail=causal):
                    f()
                flush_outq()

    nc.compile()
    return nc


_NC_CACHE = {}


def _get_nc(causal: bool):
    if causal not in _NC_CACHE:
        _NC_CACHE[causal] = _build_nc(causal)
    return _NC_CACHE[causal]


def _host_consts():
    p = np.zeros((128, 128), np.float32)
    idx = np.arange(0, 128, 2)
    p[idx, idx + 1] = -1.0
    p[idx + 1, idx] = 1.0
    psigT = np.ascontiguousarray(p.T).astype(BF)
    pm = np.where(np.arange(128)[None, :] < np.arange(128)[:, None],
                  np.float32(NEG), np.float32(0.0))
    ptneg = np.ascontiguousarray(pm.T).astype(BF)
    ident = np.eye(128, dtype=np.float32).astype(BF)
    m384 = np.zeros((128, 384), np.float32)
    m384[:, 0:128] = np.eye(128)
    m384[:, 256:384] = np.eye(128)
    return psigT, ptneg, ident, m384.astype(BF)


def _numpy_reference(hidden_states, cos, sin, attention_mask, Wq, Wk, Wv, Wo):
    """Generic-mask fallback, pure numpy port of the reference."""
    GROUPS = H // KVH

    def rope(x, c, s):
        c = c[:, None, :, :]
        s = s[:, None, :, :]
        x1, x2 = x[..., ::2], x[..., 1::2]
        xr = np.stack([x1 * c - x2 * s, x1 * s + x2 * c], axis=-1)
        return xr.reshape(x.shape)

    b, sq, d = hidden_states.shape
    q = (hidden_states @ Wq).reshape(b, sq, H, HD).transpose(0, 2, 1, 3)
    k = (hidden_states @ Wk).reshape(b, sq, KVH, HD).transpose(0, 2, 1, 3)
    v = (hidden_states @ Wv).reshape(b, sq, KVH, HD).transpose(0, 2, 1, 3)
    q = rope(q, cos, sin)
    k = rope(k, cos, sin)
    k = np.repeat(k, GROUPS, axis=1)
    v = np.repeat(v, GROUPS, axis=1)
    out = np.zeros((b, sq, d), np.float32)
    for bi in range(b):
        for hi in range(H):
            sc = (q[bi, hi] @ k[bi, hi].T) * SCALE + attention_mask[0, 0]
            sc = sc - sc.max(axis=-1, keepdims=True)
            e = np.exp(sc)
            pr = e / e.sum(axis=-1, keepdims=True)
            ctx = pr @ v[bi, hi]
            out[bi] += ctx @ Wo[hi * HD:(hi + 1) * HD]
    return out


def kernel(**inputs) -> np.ndarray:
    hs = np.asarray(inputs["hidden_states"], np.float32)
    cos = np.asarray(inputs["cos"], np.float32)
    sin = np.asarray(inputs["sin"], np.float32)
    mask = np.asarray(inputs["attention_mask"], np.float32)
    Wq = np.asarray(inputs["Wq"], np.float32)
    Wk = np.asarray(inputs["Wk"], np.float32)
    Wv = np.asarray(inputs["Wv"], np.float32)
    Wo = np.asarray(inputs["Wo"], np.float32)

    m = mask.reshape(S, S)
    tril = np.tril(np.ones((S, S), dtype=bool))
    causal_ref = np.where(tril, np.float32(0.0), np.float32(NEG))
    if np.array_equal(m, causal_ref):
        causal = True
    elif not m.any():
        causal = False
    else:
        return _numpy_reference(hs, cos, sin, mask, Wq, Wk, Wv, Wo)

    nc = _get_nc(causal)
    psigT, ptneg, ident, m384 = _host_consts()
    chan_half = (np.arange(64) // 2)

    in_maps = []
    for core in range(8):
        b, t = core // TP, core % TP
        hTf = np.ascontiguousarray(hs[b].T)                       # [D, S]
        hT = np.ascontiguousarray(
            hTf.reshape(8, 128, S).transpose(1, 0, 2)).astype(BF)
        cs64 = np.ascontiguousarray(cos[b].T[chan_half, :])       # [64, S]
        sn64 = np.ascontiguousarray(sin[b].T[chan_half, :])
        csd = np.ascontiguousarray(np.vstack([cs64, cs64])).astype(BF)
        snd = np.ascontiguousarray(np.vstack([sn64, sn64])).astype(BF)
        wq_s = np.ascontiguousarray(
            Wq[:, t * 256:(t + 1) * 256].reshape(8, 128, 256)
            .transpose(1, 0, 2)).astype(BF)
        wkv_f = np.concatenate([Wk[:, t * 64:(t + 1) * 64],
                                Wv[:, t * 64:(t + 1) * 64]], axis=1)
        wkv_s = np.ascontiguousarray(
            wkv_f.reshape(8, 128, 128).transpose(1, 0, 2)).astype(BF)
        wo_s = np.ascontiguousarray(
            Wo[t * 256:(t + 1) * 256].reshape(2, 128, D)
            .transpose(1, 0, 2)).astype(BF)
        in_maps.append({
            "hT": hT, "csd": csd, "snd": snd,
            "wq": wq_s, "wkv": wkv_s, "wo": wo_s,
            "psigT": psigT, "ptneg": ptneg, "ident": ident, "m384": m384,
        })

    res = run_bass_kernel_spmd(nc, in_maps, core_ids=list(range(8)))
    out = np.zeros((B, S, D), np.float32)
    for core in range(8):
        out[core // TP] += res.results[core]["out"].astype(np.float32)
    return out


# revision 48
# speedup vs baseline: 1.3576x; 1.0437x over previous
"""Self-contained Trainium2 Bass kernel for GQA MultiHeadAttention with RoPE.

Problem: B=2, S=2048, D=1024, H=16 Q heads, KVH=4 KV heads, head_dim=64,
causal additive mask, f32.

Sharding: tensor-parallel over heads (TP=4: 4 Q heads + 1 KV head per shard)
x data-parallel over batch (DP=2) = 8 NeuronCores. Wo is sharded on its
input dim; the host sums the 4 partial outputs per batch element.

Design notes (all bf16 datapath, f32 psum):
 - scores kept kv-major ([kv, q] psum tiles) so exp feeds strictly from PE;
   causal diagonal handled by a -1e9 mask ADDED via a PT @ I matmul into the
   same psum accumulation group (no post-exp mask multiplies).
 - ctx computed q-major: stationary = exp'd scores chunk [kv,128q], moving =
   V||ones [kv,65]  ->  psum [128q, 65].  The softmax denominator lands on
   column 64, per-partition, so normalize = reciprocal + tensor_scalar_mul,
   no DRAM round trip.
 - ctx transposed back to ch-major for the output projection with the
   DMA xbar transpose (SBUF->SBUF, bf16).
 - Activation engine runs exps only; evictions go to DVE/ACT split; GPSIMD
   cannot touch PSUM.
"""

import os
import sys

for _p in ("/opt/trn_rl_repo", "/root/.axon_site/_ro/trn_rl_repo"):
    if os.path.isdir(_p) and _p not in sys.path:
        sys.path.insert(0, _p)

import numpy as np
import ml_dtypes

import concourse.bacc as bacc
import concourse.bass as bass
import concourse.tile as tile
from concourse import mybir
from concourse.bass_utils import run_bass_kernel_spmd

F32 = mybir.dt.float32
BF16 = mybir.dt.bfloat16
F8 = mybir.dt.float8e4
DR = mybir.MatmulPerfMode.DoubleRow
WS = 16.0                   # fp8 weight pre-scale (split residual headroom)
AF = mybir.ActivationFunctionType
BF = ml_dtypes.bfloat16
F8NP = ml_dtypes.float8_e4m3fn


def _split8(x):
    """fp8 value/residual split (f32 -> two e4m3 arrays)."""
    x1 = x.astype(F8NP)
    x2 = (x - x1.astype(np.float32)).astype(F8NP)
    return x1, x2


def _pairs(x, ncols):
    """[D, N] -> [128, 4, 2, N] DoubleRow pair layout."""
    return np.ascontiguousarray(
        x.reshape(4, 2, 128, ncols).transpose(2, 0, 1, 3))

H, KVH, HD = 16, 4, 64
B, S, D = 2, 2048, 1024
TP = 4                      # head-parallel ways
SCALE = HD ** -0.5
NEG = -1e9
NT = S // 128               # 16 kv tiles
NQB = S // 512              # 4 q blocks


def _patch_act_tables():
    """Make Exp resolve only to natural_log_exp_and_others so the act-table
    pass emits a single table load."""
    from concourse.hw_specs import get_activation_tables
    t = get_activation_tables("gen3")
    for name, fns in t.items():
        if name != "natural_log_exp_and_others":
            fns.discard(AF.Exp)
            fns.discard(AF.Ln)


def _build_nc(causal: bool):
    _patch_act_tables()
    nc = bacc.Bacc()

    ht8a = nc.declare_dram_parameter("ht8a", [128, 4, 2, S], F8, isOutput=False)
    ht8b = nc.declare_dram_parameter("ht8b", [128, 4, 2, S], F8, isOutput=False)
    csd = nc.declare_dram_parameter("csd", [128, S], BF16, isOutput=False)
    snd = nc.declare_dram_parameter("snd", [128, S], BF16, isOutput=False)
    wq8a = nc.declare_dram_parameter("wq8a", [128, 4, 2, 256], F8, isOutput=False)
    wq8b = nc.declare_dram_parameter("wq8b", [128, 4, 2, 256], F8, isOutput=False)
    wkv8a = nc.declare_dram_parameter("wkv8a", [128, 4, 2, 128], F8, isOutput=False)
    wkv8b = nc.declare_dram_parameter("wkv8b", [128, 4, 2, 128], F8, isOutput=False)
    wo = nc.declare_dram_parameter("wo", [128, 2, D], BF16, isOutput=False)
    psigT = nc.declare_dram_parameter("psigT", [128, 128], BF16, isOutput=False)
    ptneg = nc.declare_dram_parameter("ptneg", [128, 128], BF16, isOutput=False)
    ident = nc.declare_dram_parameter("ident", [128, 128], BF16, isOutput=False)
    m384 = nc.declare_dram_parameter("m384", [128, 384], BF16, isOutput=False)
    outp = nc.declare_dram_parameter("out", [S, D], BF16, isOutput=True)

    with tile.TileContext(nc) as tc:
        with tc.tile_pool(name="hold", bufs=1) as hp:
            # ---- constants / weights (two DMA queues) ----
            # single sync queue, strict priority order for the first-exp path
            wkva_sb = hp.tile([128, 4, 2, 128], F8, name="wkva_sb",
                              tag="wkva_sb")
            nc.sync.dma_start(out=wkva_sb, in_=wkv8a[:, :, :, :])
            hta_sb = hp.tile([128, 4, 2, S], F8, name="hta_sb", tag="hta_sb")
            htb_sb = hp.tile([128, 4, 2, S], F8, name="htb_sb", tag="htb_sb")
            nc.sync.dma_start(out=hta_sb[:, :, :, 0:512],
                              in_=ht8a[:, :, :, 0:512])
            wkvb_sb = hp.tile([128, 4, 2, 128], F8, name="wkvb_sb",
                              tag="wkvb_sb")
            nc.sync.dma_start(out=wkvb_sb, in_=wkv8b[:, :, :, :])
            nc.sync.dma_start(out=htb_sb[:, :, :, 0:512],
                              in_=ht8b[:, :, :, 0:512])
            cos_sb = hp.tile([128, S], BF16, name="cos_sb", tag="cos_sb")
            sin_sb = hp.tile([128, S], BF16, name="sin_sb", tag="sin_sb")
            nc.sync.dma_start(out=cos_sb[:, 0:512], in_=csd[:, 0:512])
            nc.sync.dma_start(out=sin_sb[:, 0:512], in_=snd[:, 0:512])
            psig_sb = hp.tile([128, 128], BF16, name="psig_sb", tag="psig_sb")
            nc.sync.dma_start(out=psig_sb, in_=psigT[:, :])
            pt_sb = hp.tile([128, 128], BF16, name="pt_sb", tag="pt_sb")
            nc.sync.dma_start(out=pt_sb, in_=ptneg[:, :])
            id_sb = hp.tile([128, 128], BF16, name="id_sb", tag="id_sb")
            nc.sync.dma_start(out=id_sb, in_=ident[:, :])
            m384_sb = hp.tile([128, 384], BF16, name="m384_sb", tag="m384_sb")
            nc.sync.dma_start(out=m384_sb, in_=m384[:, :])
            wqa_sb = hp.tile([128, 4, 2, 256], F8, name="wqa_sb", tag="wqa_sb")
            nc.sync.dma_start(out=wqa_sb, in_=wq8a[:, :, :, :])
            wqb_sb = hp.tile([128, 4, 2, 256], F8, name="wqb_sb", tag="wqb_sb")
            nc.sync.dma_start(out=wqb_sb, in_=wq8b[:, :, :, :])
            nc.sync.dma_start(out=hta_sb[:, :, :, 512:1024],
                              in_=ht8a[:, :, :, 512:1024])
            nc.sync.dma_start(out=htb_sb[:, :, :, 512:1024],
                              in_=ht8b[:, :, :, 512:1024])
            nc.sync.dma_start(out=cos_sb[:, 512:S], in_=csd[:, 512:S])
            nc.sync.dma_start(out=sin_sb[:, 512:S], in_=snd[:, 512:S])
            for sc in range(2, 4):
                csl = slice(512 * sc, 512 * sc + 512)
                nc.sync.dma_start(out=hta_sb[:, :, :, csl],
                                  in_=ht8a[:, :, :, csl])
                nc.sync.dma_start(out=htb_sb[:, :, :, csl],
                                  in_=ht8b[:, :, :, csl])
            wo_sb = hp.tile([128, 2, D], BF16, name="wo_sb", tag="wo_sb")
            nc.sync.dma_start(out=wo_sb, in_=wo[:, :, :])

            qTs = [hp.tile([128, S], BF16, name=f"qT{p}", tag=f"qT{p}")
                   for p in range(2)]
            kT = hp.tile([128, S], BF16, name="kTt", tag="kTt")
            vsm = hp.tile([128, NT, 65], BF16, name="vsm", tag="vsm")
            nc.gpsimd.memset(vsm[:, :, 64:65], 1.0)
            zer_sb = hp.tile([128, 512], BF16, name="zer_sb", tag="zer_sb")
            nc.gpsimd.memset(zer_sb, 0.0)

            with tc.tile_pool(name="psS", bufs=1, space="PSUM") as psS, \
                 tc.tile_pool(name="psD", bufs=1, space="PSUM") as psD, \
                 tc.tile_pool(name="psQ", bufs=1, space="PSUM") as psQ, \
                 tc.tile_pool(name="etp", bufs=1) as etp, \
                 tc.tile_pool(name="sbA", bufs=1) as sbA, \
                 tc.tile_pool(name="sbC", bufs=1) as sbC:

                # per-qb rotating ctx tiles (q-major and transposed ch-major)
                # A: heads 0,1 (ch 0:128); B: heads 2,3 (ch 128:256)
                ctxq = [sbC.tile([128, 4, 128], BF16, name=f"ctxq{h2}",
                                 tag=f"ctxq{h2}", bufs=2) for h2 in range(2)]
                ctxT = [sbC.tile([128, 4, 128], BF16, name=f"ctxT{h2}",
                                 tag=f"ctxT{h2}", bufs=2) for h2 in range(2)]

                state = {"ctxq": [ctxq[0], ctxq[1]],
                         "ctxT": [ctxT[0], ctxT[1]],
                         "outq": []}

                def flush_outq():
                    for dst, ost in state["outq"]:
                        nc.sync.dma_start(out=dst, in_=ost)
                    state["outq"] = []

                # ---------------- Phase A: projections + rope ----------------
                # Emitted as ~0.5us micro-steps so interleaving into the BC
                # unit stream never starves the exp cadence.
                def a_psum(tag):
                    # prologue can borrow the (idle) score banks
                    if tag == "ps_s":
                        return psS.tile([128, 1024], F32, name="ps_a",
                                        tag="ps_s", bufs=2)[:, 0:512]
                    return psD.tile([128, 512], F32, name="ps_a",
                                    tag="ps_d", bufs=2)

                def steps_kv(sc, tag="ps_d"):
                    csl = slice(512 * sc, 512 * sc + 512)
                    box = {}

                    # 3-term fp8 DoubleRow split: a1*w1, a1*w2, a2*w1
                    terms = [(hta_sb, wkva_sb), (hta_sb, wkvb_sb),
                             (htb_sb, wkva_sb)]

                    def proj(ti):
                        if ti == 0:
                            box["ps"] = a_psum(tag)
                            box["kvraw"] = sbA.tile([128, 512], BF16,
                                                    name="kvraw", tag="kvraw",
                                                    bufs=2)
                        a_t, w_t = terms[ti]
                        for pr in range(4):
                            nc.tensor.matmul(box["ps"], w_t[:, pr, :, :],
                                             a_t[:, pr, :, csl],
                                             start=(ti == 0 and pr == 0),
                                             stop=(ti == 2 and pr == 3),
                                             perf_mode=DR)
                        if ti == 2:
                            nc.vector.tensor_copy(box["kvraw"], box["ps"])

                    def krot():
                        ps_kr = a_psum(tag)[0:64, :]
                        kvraw = box["kvraw"]
                        nc.tensor.matmul(ps_kr, psig_sb[0:64, 0:64],
                                         kvraw[0:64, :], start=True,
                                         stop=True)
                        kdst = kT[0:64, csl]
                        nc.vector.tensor_mul(kdst, kvraw[0:64, :],
                                             cos_sb[0:64, csl])
                        ktmp = sbA.tile([64, 512], BF16, name="ktmp",
                                        tag="ktmp", bufs=2)
                        nc.vector.tensor_mul(ktmp, ps_kr, sin_sb[0:64, csl])
                        nc.vector.tensor_add(kdst, kdst, ktmp)

                    def vt(tt):
                        ti = 4 * sc + tt
                        ps_v = a_psum(tag)
                        ps_vb = ps_v.bitcast(BF16)[:, 0:64]
                        nc.tensor.matmul(
                            ps_vb,
                            box["kvraw"][64:128, 128 * tt:128 * tt + 128],
                            id_sb[64:128, 64:128],
                            start=True, stop=True, is_transpose=True)
                        nc.vector.tensor_scalar_mul(vsm[:, ti, 0:64],
                                                    ps_vb, 1.0 / WS)

                    return ([lambda ti=ti: proj(ti) for ti in range(3)]
                            + [krot]
                            + [lambda tt=tt: vt(tt) for tt in range(4)])

                def steps_q(sc, pp, tag="ps_d"):
                    csl = slice(512 * sc, 512 * sc + 512)
                    box = {}

                    terms = [(hta_sb, wqa_sb), (hta_sb, wqb_sb),
                             (htb_sb, wqa_sb)]

                    def proj(ti):
                        if ti == 0:
                            if pp == 1:
                                # deferred K duplicate (waits settled by now)
                                nc.sync.dma_start(out=kT[64:128, csl],
                                                  in_=kT[0:64, csl])
                            box["ps"] = a_psum(tag)
                            box["qraw"] = sbA.tile([128, 512], BF16,
                                                   name="qraw", tag="qraw",
                                                   bufs=2)
                        a_t, w_t = terms[ti]
                        for pr in range(4):
                            nc.tensor.matmul(
                                box["ps"],
                                w_t[:, pr, :, 128 * pp:128 * pp + 128],
                                a_t[:, pr, :, csl],
                                start=(ti == 0 and pr == 0),
                                stop=(ti == 2 and pr == 3),
                                perf_mode=DR)
                        if ti == 2:
                            nc.vector.tensor_copy(box["qraw"], box["ps"])

                    def qrot():
                        qraw = box["qraw"]
                        ps_r = a_psum(tag)
                        nc.tensor.matmul(ps_r, psig_sb, qraw, start=True,
                                         stop=True)
                        dst = qTs[pp][:, csl]
                        nc.vector.tensor_mul(dst, qraw, cos_sb[:, csl])
                        rtmp = sbA.tile([128, 512], BF16, name="rtmp",
                                        tag="rtmp", bufs=2)
                        nc.vector.tensor_mul(rtmp, ps_r, sin_sb[:, csl])
                        nc.vector.tensor_add(dst, dst, rtmp)

                    return ([lambda ti=ti: proj(ti) for ti in range(3)]
                            + [qrot])

                # ---------------- Phase BC: attention ----------------
                def emit_bc(qb, filler):
                    """Attention for q block qb, 4 heads; unit-pipelined.

                    filler: list of closures emitting independent PE work,
                    popped between units to cover exp latency.
                    """
                    qsl = slice(512 * qb, 512 * qb + 512)
                    nfull = 4 * qb if causal else NT
                    pending = []
                    n_units = 4 * ((nfull + 1) // 2 + (2 if causal else 0))
                    bstate = {"left": max(n_units, 1), "carry": 0.0}

                    def boundary(flush=False):
                        # deferred emissions first (their waits are settled),
                        # then evenly-paced independent PE filler work
                        for _ in range(len(pending)):
                            pending.pop(0)()
                        if flush:
                            n = len(filler)
                        else:
                            bstate["carry"] += len(filler) / bstate["left"]
                            n = int(bstate["carry"])
                            bstate["carry"] -= n
                            bstate["left"] = max(bstate["left"] - 1, 1)
                        for _ in range(n):
                            if filler:
                                filler.pop(0)()

                    for h in range(4):
                        off = 64 * (h % 2)
                        pp = h // 2
                        ps_qm = psQ.tile([128, 4, 128], F32, name="ps_qm",
                                         tag="ps_qm", bufs=2)
                        # one accumulation group per psum bank: zero the whole
                        # bank once, then every ctx matmul accumulates.  After
                        # the first per-qc stop the bank flag is cleared, so
                        # later accumulates skip the group check.
                        nc.tensor.matmul(ps_qm[:, :, :].opt(), pt_sb, zer_sb,
                                         start=True, stop=False)
                        hstate = {"stopped": False}

                        def ctx_mm(ki, et_ap, qc):
                            # ctx[qc] += et_chunk.T @ (V||1)
                            last = (4 * qb + qc) if causal else (NT - 1)
                            stop = (ki == last)
                            nc.tensor.matmul(
                                ps_qm[:, qc, 0:65], et_ap, vsm[:, ki, :],
                                start=False, stop=stop,
                                skip_group_check=hstate["stopped"])
                            if stop:
                                hstate["stopped"] = True

                        units = []

                        def mk_pair(kp, _off=off, _pp=pp):
                            box = {}

                            def s():
                                ps_s = psS.tile([128, 1024], F32, name="ps_s",
                                                tag="ps_s", bufs=2)
                                for jj in range(2):
                                    ki = kp + jj
                                    nc.tensor.matmul(
                                        ps_s[:, 512 * jj:512 * jj + 512],
                                        kT[_off:_off + 64,
                                           128 * ki:128 * ki + 128],
                                        qTs[_pp][_off:_off + 64, qsl],
                                        start=True, stop=True)
                                box["ps"] = ps_s

                            def ec():
                                et = etp.tile([128, 1024], BF16, name="et",
                                              tag="et", bufs=4)
                                nc.scalar.activation(et, box["ps"], AF.Exp,
                                                     scale=SCALE)
                                for qc in range(4):
                                    for jj in range(2):
                                        ctx_mm(kp + jj,
                                               et[:, 512 * jj + 128 * qc:
                                                  512 * jj + 128 * qc + 128],
                                               qc)
                            return (s, ec)

                        def mk_diagA(_off=off, _pp=pp, _qb=qb):
                            # kv tiles 4qb+0 (span 512) and 4qb+1 (span 384)
                            box = {}
                            k0 = 4 * _qb

                            def s():
                                ps = psS.tile([128, 1024], F32, name="ps_dA",
                                              tag="ps_s", bufs=2)
                                nc.tensor.matmul(
                                    ps[:, 0:512],
                                    kT[_off:_off + 64,
                                       128 * k0:128 * k0 + 128],
                                    qTs[_pp][_off:_off + 64, qsl],
                                    start=True, stop=False)
                                nc.tensor.matmul(
                                    ps[:, 0:128], pt_sb, id_sb,
                                    start=False, stop=True)
                                nc.tensor.matmul(
                                    ps[:, 512:896],
                                    kT[_off:_off + 64,
                                       128 * k0 + 128:128 * k0 + 256],
                                    qTs[_pp][_off:_off + 64,
                                             512 * _qb + 128:512 * _qb + 512],
                                    start=True, stop=False)
                                nc.tensor.matmul(
                                    ps[:, 512:640], pt_sb, id_sb,
                                    start=False, stop=True)
                                box["ps"] = ps

                            def ec():
                                et = etp.tile([128, 896], BF16, name="etdA",
                                              tag="etdA", bufs=2)
                                nc.scalar.activation(et, box["ps"][:, 0:896],
                                                     AF.Exp, scale=SCALE)
                                for qc in range(4):
                                    ctx_mm(k0, et[:, 128 * qc:128 * qc + 128],
                                           qc)
                                for qc in range(1, 4):
                                    ctx_mm(k0 + 1,
                                           et[:, 512 + 128 * (qc - 1):
                                              512 + 128 * qc], qc)
                            return (s, ec)

                        def mk_diagB(_off=off, _pp=pp, _qb=qb):
                            # kv tiles 4qb+2 (span 256) and 4qb+3 (span 128)
                            box = {}
                            k2 = 4 * _qb + 2

                            def s():
                                ps = psS.tile([128, 1024], F32, name="ps_dB",
                                              tag="ps_s", bufs=2)
                                # one group per bank: the [P|0|P] mask
                                # preload writes (and zeroes) every byte the
                                # exp will read, then scores accumulate
                                nc.tensor.matmul(
                                    ps[:, 0:384], pt_sb, m384_sb,
                                    start=True, stop=False)
                                nc.tensor.matmul(
                                    ps[:, 0:256],
                                    kT[_off:_off + 64,
                                       128 * k2:128 * k2 + 128],
                                    qTs[_pp][_off:_off + 64,
                                             512 * _qb + 256:512 * _qb + 512],
                                    start=False, stop=False)
                                nc.tensor.matmul(
                                    ps[:, 256:384],
                                    kT[_off:_off + 64,
                                       128 * k2 + 128:128 * k2 + 256],
                                    qTs[_pp][_off:_off + 64,
                                             512 * _qb + 384:512 * _qb + 512],
                                    start=False, stop=True)
                                box["ps"] = ps

                            def ec():
                                et = etp.tile([128, 384], BF16, name="etdB",
                                              tag="etdB", bufs=2)
                                nc.scalar.activation(et, box["ps"][:, 0:384],
                                                     AF.Exp, scale=SCALE)
                                for qc in range(2, 4):
                                    ctx_mm(k2, et[:, 128 * (qc - 2):
                                                  128 * (qc - 1)], qc)
                                ctx_mm(k2 + 1, et[:, 256:384], 3)
                            return (s, ec)

                        for kp in range(0, nfull, 2):
                            units.append(mk_pair(kp))
                        if causal:
                            units.append(mk_diagA())
                            units.append(mk_diagB())

                        # normalize + evict q-major ctx (bf16); deferred one
                        # unit so the semaphore waits never clog DVE's queue
                        def normalize(_h=h, _ps=ps_qm, _qb=qb, qcs=(0, 1, 2, 3),
                                      tail_h=False):
                            rcp = sbC.tile([128, len(qcs)], F32, name="rcp",
                                           tag="rcp", bufs=4)
                            nc.vector.reciprocal_approx_fast(
                                rcp, _ps[:, qcs[0]:qcs[-1] + 1, 64:65].opt())
                            dstq = state["ctxq"][_h // 2]
                            dstT = state["ctxT"][_h // 2]
                            col = 64 * (_h % 2)
                            for i, qc in enumerate(qcs):
                                nc.vector.tensor_scalar_mul(
                                    dstq[:, qc, col:col + 64],
                                    _ps[:, qc, 0:64], rcp[:, i:i + 1])
                                if tail_h:
                                    # progressive per-qc transpose right
                                    # behind each normalize chunk, on
                                    # alternating queues
                                    eng = nc.sync if qc % 2 == 0 else nc.scalar
                                    eng.dma_start_transpose(
                                        dstT[:, qc, :], dstq[:, qc, :])
                            if _h % 2 == 1 and not tail_h:
                                def transpose(_h2=_h // 2):
                                    dT = state["ctxT"][_h2]
                                    dq = state["ctxq"][_h2]
                                    nc.sync.dma_start_transpose(
                                        dT[:, :, :], dq[:, :, :])
                                pending.append(transpose)

                        is_tail_h = (causal and qb == NQB - 1 and h == 3)
                        if units:
                            units[0][0]()
                        for i in range(len(units)):
                            if i + 1 < len(units):
                                units[i + 1][0]()
                            boundary()
                            units[i][1]()
                            if is_tail_h and i == len(units) - 2:
                                # qc0/1 groups stopped in diagA: normalize
                                # them while diagB is still in flight
                                pending.append(
                                    lambda: normalize(qcs=(0, 1),
                                                      tail_h=True))

                        if is_tail_h:
                            pending.append(lambda: normalize(qcs=(2, 3),
                                                             tail_h=True))
                        else:
                            pending.append(normalize)
                    while pending:
                        pending.pop(0)()
                    boundary(flush=True)

                # ---------------- Phase D: output projection ----------------
                def emit_d(qb, ctxTA, ctxTB, tail=False):
                    out = []
                    for qt in range(4):
                        for nb in range(2):
                            def f(_qt=qt, _nb=nb, _qb=qb, _A=ctxTA, _B=ctxTB):
                                ps_o = psD.tile([128, 512], F32, name="ps_o",
                                                tag="ps_d", bufs=2)
                                nsl = slice(512 * _nb, 512 * _nb + 512)
                                nc.tensor.matmul(ps_o, _A[:, _qt, :],
                                                 wo_sb[:, 0, nsl],
                                                 start=True, stop=False)
                                nc.tensor.matmul(ps_o, _B[:, _qt, :],
                                                 wo_sb[:, 1, nsl],
                                                 start=False, stop=True)
                                ost = sbC.tile([128, 512], BF16, name="ost",
                                               tag="ost", bufs=16)
                                if tail and _nb == 1:
                                    nc.scalar.copy(ost, ps_o)
                                else:
                                    nc.vector.tensor_copy(ost, ps_o)
                                dst = outp[512 * _qb + 128 * _qt:
                                           512 * _qb + 128 * _qt + 128, nsl]
                                if tail:
                                    nc.sync.dma_start(out=dst, in_=ost)
                                else:
                                    # deferred so its wait is settled by the
                                    # time it hits the SP queue
                                    state["outq"].append((dst, ost))
                            out.append(f)
                    return out

                # ---------------- global schedule ----------------
                def afiller(sc):
                    return (steps_kv(sc) + steps_q(sc, 0) + steps_q(sc, 1))

                def rotate_ctx():
                    state["ctxq"] = [
                        sbC.tile([128, 4, 128], BF16, name=f"ctxq{h2}",
                                 tag=f"ctxq{h2}", bufs=2) for h2 in range(2)]
                    state["ctxT"] = [
                        sbC.tile([128, 4, 128], BF16, name=f"ctxT{h2}",
                                 tag=f"ctxT{h2}", bufs=2) for h2 in range(2)]

                if causal:
                    # wavefront: BC(qb) only needs kv/q chunks <= qb.
                    # Prologue: kv proj -> q proj (borrowing score banks) ->
                    # k rope -> q rope; V transposes land in BC(0)'s filler.
                    kv0 = steps_kv(0)
                    q00 = steps_q(0, 0, tag="ps_s")
                    for st in kv0[0:3]:         # kv proj
                        st()
                    for st in q00[0:3]:         # q proj (ps_s banks)
                        st()
                    kv0[3]()                    # k rope
                    q00[3]()                    # q rope
                    emit_bc(0, kv0[4:8] + steps_q(0, 1) + afiller(1))
                    for qb in range(1, NQB):
                        dA = emit_d(qb - 1, state["ctxT"][0],
                                    state["ctxT"][1])
                        rotate_ctx()
                        fill = (afiller(qb + 1) if qb + 1 < NQB else [])
                        fill += [flush_outq] + dA
                        emit_bc(qb, fill)
                else:
                    # full attention needs all kv before any BC
                    for sc in range(4):
                        for st in steps_kv(sc) + steps_q(sc, 0) \
                                + steps_q(sc, 1):
                            st()
                    emit_bc(0, [])
                    for qb in range(1, NQB):
                        dA = emit_d(qb - 1, state["ctxT"][0],
                                    state["ctxT"][1])
                        rotate_ctx()
                        emit_bc(qb, [flush_outq] + dA)
                for f in emit_d(NQB - 1, state["ctxT"][0], state["ctxT"][1],
                                tail=causal):
                    f()
                flush_outq()

    nc.compile()
    return nc


_NC_CACHE = {}


def _get_nc(causal: bool):
    if causal not in _NC_CACHE:
        _NC_CACHE[causal] = _build_nc(causal)
    return _NC_CACHE[causal]


def _host_consts():
    p = np.zeros((128, 128), np.float32)
    idx = np.arange(0, 128, 2)
    p[idx, idx + 1] = -1.0
    p[idx + 1, idx] = 1.0
    psigT = np.ascontiguousarray(p.T).astype(BF)
    pm = np.where(np.arange(128)[None, :] < np.arange(128)[:, None],
                  np.float32(NEG), np.float32(0.0))
    ptneg = np.ascontiguousarray(pm.T).astype(BF)
    ident = np.eye(128, dtype=np.float32).astype(BF)
    m384 = np.zeros((128, 384), np.float32)
    m384[:, 0:128] = np.eye(128)
    m384[:, 256:384] = np.eye(128)
    return psigT, ptneg, ident, m384.astype(BF)


def _numpy_reference(hidden_states, cos, sin, attention_mask, Wq, Wk, Wv, Wo):
    """Generic-mask fallback, pure numpy port of the reference."""
    GROUPS = H // KVH

    def rope(x, c, s):
        c = c[:, None, :, :]
        s = s[:, None, :, :]
        x1, x2 = x[..., ::2], x[..., 1::2]
        xr = np.stack([x1 * c - x2 * s, x1 * s + x2 * c], axis=-1)
        return xr.reshape(x.shape)

    b, sq, d = hidden_states.shape
    q = (hidden_states @ Wq).reshape(b, sq, H, HD).transpose(0, 2, 1, 3)
    k = (hidden_states @ Wk).reshape(b, sq, KVH, HD).transpose(0, 2, 1, 3)
    v = (hidden_states @ Wv).reshape(b, sq, KVH, HD).transpose(0, 2, 1, 3)
    q = rope(q, cos, sin)
    k = rope(k, cos, sin)
    k = np.repeat(k, GROUPS, axis=1)
    v = np.repeat(v, GROUPS, axis=1)
    out = np.zeros((b, sq, d), np.float32)
    for bi in range(b):
        for hi in range(H):
            sc = (q[bi, hi] @ k[bi, hi].T) * SCALE + attention_mask[0, 0]
            sc = sc - sc.max(axis=-1, keepdims=True)
            e = np.exp(sc)
            pr = e / e.sum(axis=-1, keepdims=True)
            ctx = pr @ v[bi, hi]
            out[bi] += ctx @ Wo[hi * HD:(hi + 1) * HD]
    return out


def kernel(**inputs) -> np.ndarray:
    hs = np.asarray(inputs["hidden_states"], np.float32)
    cos = np.asarray(inputs["cos"], np.float32)
    sin = np.asarray(inputs["sin"], np.float32)
    mask = np.asarray(inputs["attention_mask"], np.float32)
    Wq = np.asarray(inputs["Wq"], np.float32)
    Wk = np.asarray(inputs["Wk"], np.float32)
    Wv = np.asarray(inputs["Wv"], np.float32)
    Wo = np.asarray(inputs["Wo"], np.float32)

    m = mask.reshape(S, S)
    tril = np.tril(np.ones((S, S), dtype=bool))
    causal_ref = np.where(tril, np.float32(0.0), np.float32(NEG))
    if np.array_equal(m, causal_ref):
        causal = True
    elif not m.any():
        causal = False
    else:
        return _numpy_reference(hs, cos, sin, mask, Wq, Wk, Wv, Wo)

    nc = _get_nc(causal)
    psigT, ptneg, ident, m384 = _host_consts()
    chan_half = (np.arange(64) // 2)

    in_maps = []
    for core in range(8):
        b, t = core // TP, core % TP
        hTf = np.ascontiguousarray(hs[b].T)                       # [D, S]
        h1, h2 = _split8(hTf)
        ht8a = _pairs(h1, S)
        ht8b = _pairs(h2, S)
        cs64 = np.ascontiguousarray(cos[b].T[chan_half, :])       # [64, S]
        sn64 = np.ascontiguousarray(sin[b].T[chan_half, :])
        csd = (np.vstack([cs64, cs64]) / WS).astype(BF)
        snd = (np.vstack([sn64, sn64]) / WS).astype(BF)
        wq1, wq2 = _split8(Wq[:, t * 256:(t + 1) * 256] * WS)
        wkv_f = np.concatenate([Wk[:, t * 64:(t + 1) * 64],
                                Wv[:, t * 64:(t + 1) * 64]], axis=1)
        wk1, wk2 = _split8(wkv_f * WS)
        wo_s = np.ascontiguousarray(
            Wo[t * 256:(t + 1) * 256].reshape(2, 128, D)
            .transpose(1, 0, 2)).astype(BF)
        in_maps.append({
            "ht8a": ht8a, "ht8b": ht8b, "csd": csd, "snd": snd,
            "wq8a": _pairs(wq1, 256), "wq8b": _pairs(wq2, 256),
            "wkv8a": _pairs(wk1, 128), "wkv8b": _pairs(wk2, 128),
            "wo": wo_s,
            "psigT": psigT, "ptneg": ptneg, "ident": ident, "m384": m384,
        })

    res = run_bass_kernel_spmd(nc, in_maps, core_ids=list(range(8)))
    out = np.zeros((B, S, D), np.float32)
    for core in range(8):
        out[core // TP] += res.results[core]["out"].astype(np.float32)
    return out


# revision 56
# speedup vs baseline: 1.3593x; 1.0013x over previous
"""Self-contained Trainium2 Bass kernel for GQA MultiHeadAttention with RoPE.

Problem: B=2, S=2048, D=1024, H=16 Q heads, KVH=4 KV heads, head_dim=64,
causal additive mask, f32.

Sharding: tensor-parallel over heads (TP=4: 4 Q heads + 1 KV head per shard)
x data-parallel over batch (DP=2) = 8 NeuronCores. Wo is sharded on its
input dim; the host sums the 4 partial outputs per batch element.

Design notes (all bf16 datapath, f32 psum):
 - scores kept kv-major ([kv, q] psum tiles) so exp feeds strictly from PE;
   causal diagonal handled by a -1e9 mask ADDED via a PT @ I matmul into the
   same psum accumulation group (no post-exp mask multiplies).
 - ctx computed q-major: stationary = exp'd scores chunk [kv,128q], moving =
   V||ones [kv,65]  ->  psum [128q, 65].  The softmax denominator lands on
   column 64, per-partition, so normalize = reciprocal + tensor_scalar_mul,
   no DRAM round trip.
 - ctx transposed back to ch-major for the output projection with the
   DMA xbar transpose (SBUF->SBUF, bf16).
 - Activation engine runs exps only; evictions go to DVE/ACT split; GPSIMD
   cannot touch PSUM.
"""

import os
import sys

for _p in ("/opt/trn_rl_repo", "/root/.axon_site/_ro/trn_rl_repo"):
    if os.path.isdir(_p) and _p not in sys.path:
        sys.path.insert(0, _p)

import numpy as np
import ml_dtypes

import concourse.bacc as bacc
import concourse.bass as bass
import concourse.tile as tile
from concourse import mybir
from concourse.bass_utils import run_bass_kernel_spmd

F32 = mybir.dt.float32
BF16 = mybir.dt.bfloat16
F8 = mybir.dt.float8e4
DR = mybir.MatmulPerfMode.DoubleRow
WS = 16.0                   # fp8 weight pre-scale (split residual headroom)
AF = mybir.ActivationFunctionType
BF = ml_dtypes.bfloat16
F8NP = ml_dtypes.float8_e4m3fn


def _split8(x):
    """fp8 value/residual split (f32 -> two e4m3 arrays)."""
    x1 = x.astype(F8NP)
    x2 = (x - x1.astype(np.float32)).astype(F8NP)
    return x1, x2


def _pairs(x, ncols):
    """[D, N] -> [128, 4, 2, N] DoubleRow pair layout."""
    return np.ascontiguousarray(
        x.reshape(4, 2, 128, ncols).transpose(2, 0, 1, 3))

H, KVH, HD = 16, 4, 64
B, S, D = 2, 2048, 1024
TP = 4                      # head-parallel ways
SCALE = HD ** -0.5
NEG = -1e9
NT = S // 128               # 16 kv tiles
NQB = S // 512              # 4 q blocks


def _patch_act_tables():
    """Make Exp resolve only to natural_log_exp_and_others so the act-table
    pass emits a single table load."""
    from concourse.hw_specs import get_activation_tables
    t = get_activation_tables("gen3")
    for name, fns in t.items():
        if name != "natural_log_exp_and_others":
            fns.discard(AF.Exp)
            fns.discard(AF.Ln)


def _build_nc(causal: bool):
    _patch_act_tables()
    nc = bacc.Bacc()

    ht8a = nc.declare_dram_parameter("ht8a", [128, 4, 2, S], F8, isOutput=False)
    ht8b = nc.declare_dram_parameter("ht8b", [128, 4, 2, S], F8, isOutput=False)
    csd = nc.declare_dram_parameter("csd", [128, S], BF16, isOutput=False)
    snd = nc.declare_dram_parameter("snd", [128, S], BF16, isOutput=False)
    wq8a = nc.declare_dram_parameter("wq8a", [128, 4, 2, 256], F8, isOutput=False)
    wq8b = nc.declare_dram_parameter("wq8b", [128, 4, 2, 256], F8, isOutput=False)
    wkv8a = nc.declare_dram_parameter("wkv8a", [128, 4, 2, 128], F8, isOutput=False)
    wkv8b = nc.declare_dram_parameter("wkv8b", [128, 4, 2, 128], F8, isOutput=False)
    wo = nc.declare_dram_parameter("wo", [128, 2, D], BF16, isOutput=False)
    psigT = nc.declare_dram_parameter("psigT", [128, 128], BF16, isOutput=False)
    ptneg = nc.declare_dram_parameter("ptneg", [128, 128], BF16, isOutput=False)
    ident = nc.declare_dram_parameter("ident", [128, 128], BF16, isOutput=False)
    m384 = nc.declare_dram_parameter("m384", [128, 384], BF16, isOutput=False)
    outp = nc.declare_dram_parameter("out", [S, D], BF16, isOutput=True)

    with tile.TileContext(nc) as tc:
        with tc.tile_pool(name="hold", bufs=1) as hp:
            # ---- constants / weights (two DMA queues) ----
            # single sync queue, strict priority order for the first-exp path
            wkva_sb = hp.tile([128, 4, 2, 128], F8, name="wkva_sb",
                              tag="wkva_sb")
            nc.sync.dma_start(out=wkva_sb, in_=wkv8a[:, :, :, :])
            hta_sb = hp.tile([128, 4, 2, S], F8, name="hta_sb", tag="hta_sb")
            htb_sb = hp.tile([128, 4, 2, S], F8, name="htb_sb", tag="htb_sb")
            nc.sync.dma_start(out=hta_sb[:, :, :, 0:512],
                              in_=ht8a[:, :, :, 0:512])
            wkvb_sb = hp.tile([128, 4, 2, 128], F8, name="wkvb_sb",
                              tag="wkvb_sb")
            nc.sync.dma_start(out=wkvb_sb, in_=wkv8b[:, :, :, :])
            nc.sync.dma_start(out=htb_sb[:, :, :, 0:512],
                              in_=ht8b[:, :, :, 0:512])
            cos_sb = hp.tile([128, S], BF16, name="cos_sb", tag="cos_sb")
            sin_sb = hp.tile([128, S], BF16, name="sin_sb", tag="sin_sb")
            nc.sync.dma_start(out=cos_sb[:, 0:512], in_=csd[:, 0:512])
            nc.sync.dma_start(out=sin_sb[:, 0:512], in_=snd[:, 0:512])
            psig_sb = hp.tile([128, 128], BF16, name="psig_sb", tag="psig_sb")
            nc.sync.dma_start(out=psig_sb, in_=psigT[:, :])
            pt_sb = hp.tile([128, 128], BF16, name="pt_sb", tag="pt_sb")
            nc.sync.dma_start(out=pt_sb, in_=ptneg[:, :])
            id_sb = hp.tile([128, 128], BF16, name="id_sb", tag="id_sb")
            nc.sync.dma_start(out=id_sb, in_=ident[:, :])
            m384_sb = hp.tile([128, 384], BF16, name="m384_sb", tag="m384_sb")
            nc.sync.dma_start(out=m384_sb, in_=m384[:, :])
            wqa_sb = hp.tile([128, 4, 2, 256], F8, name="wqa_sb", tag="wqa_sb")
            nc.sync.dma_start(out=wqa_sb, in_=wq8a[:, :, :, :])
            wqb_sb = hp.tile([128, 4, 2, 256], F8, name="wqb_sb", tag="wqb_sb")
            nc.sync.dma_start(out=wqb_sb, in_=wq8b[:, :, :, :])
            nc.sync.dma_start(out=hta_sb[:, :, :, 512:1024],
                              in_=ht8a[:, :, :, 512:1024])
            nc.sync.dma_start(out=htb_sb[:, :, :, 512:1024],
                              in_=ht8b[:, :, :, 512:1024])
            nc.sync.dma_start(out=cos_sb[:, 512:S], in_=csd[:, 512:S])
            nc.sync.dma_start(out=sin_sb[:, 512:S], in_=snd[:, 512:S])
            for sc in range(2, 4):
                csl = slice(512 * sc, 512 * sc + 512)
                nc.sync.dma_start(out=hta_sb[:, :, :, csl],
                                  in_=ht8a[:, :, :, csl])
                nc.sync.dma_start(out=htb_sb[:, :, :, csl],
                                  in_=ht8b[:, :, :, csl])
            wo_sb = hp.tile([128, 2, D], BF16, name="wo_sb", tag="wo_sb")
            nc.sync.dma_start(out=wo_sb, in_=wo[:, :, :])

            qTs = [hp.tile([128, S], BF16, name=f"qT{p}", tag=f"qT{p}")
                   for p in range(2)]
            kT = hp.tile([128, S], BF16, name="kTt", tag="kTt")
            vsm = hp.tile([128, NT, 65], BF16, name="vsm", tag="vsm")
            nc.gpsimd.memset(vsm[:, :, 64:65], 1.0)
            zer_sb = hp.tile([128, 512], BF16, name="zer_sb", tag="zer_sb")
            nc.gpsimd.memset(zer_sb, 0.0)

            with tc.tile_pool(name="psS", bufs=1, space="PSUM") as psS, \
                 tc.tile_pool(name="psD", bufs=1, space="PSUM") as psD, \
                 tc.tile_pool(name="psQ", bufs=1, space="PSUM") as psQ, \
                 tc.tile_pool(name="etp", bufs=1) as etp, \
                 tc.tile_pool(name="sbA", bufs=1) as sbA, \
                 tc.tile_pool(name="sbC", bufs=1) as sbC:

                # per-qb rotating ctx tiles (q-major and transposed ch-major)
                # A: heads 0,1 (ch 0:128); B: heads 2,3 (ch 128:256)
                ctxq = [sbC.tile([128, 4, 128], BF16, name=f"ctxq{h2}",
                                 tag=f"ctxq{h2}", bufs=2) for h2 in range(2)]
                ctxT = [sbC.tile([128, 4, 128], BF16, name=f"ctxT{h2}",
                                 tag=f"ctxT{h2}", bufs=2) for h2 in range(2)]

                state = {"ctxq": [ctxq[0], ctxq[1]],
                         "ctxT": [ctxT[0], ctxT[1]],
                         "outq": []}

                def flush_outq():
                    for dst, ost in state["outq"]:
                        nc.sync.dma_start(out=dst, in_=ost)
                    state["outq"] = []

                # ---------------- Phase A: projections + rope ----------------
                # Emitted as ~0.5us micro-steps so interleaving into the BC
                # unit stream never starves the exp cadence.
                def a_psum(tag):
                    # prologue can borrow the (idle) score banks
                    if tag == "ps_s":
                        return psS.tile([128, 1024], F32, name="ps_a",
                                        tag="ps_s", bufs=2)[:, 0:512]
                    return psD.tile([128, 512], F32, name="ps_a",
                                    tag="ps_d", bufs=2)

                def steps_kv(sc, tag="ps_d", ev=None):
                    evc = (nc.scalar.copy if ev is nc.scalar
                           else nc.vector.tensor_copy)
                    csl = slice(512 * sc, 512 * sc + 512)
                    box = {}

                    # 3-term fp8 DoubleRow split: a1*w1, a1*w2, a2*w1
                    terms = [(hta_sb, wkva_sb), (hta_sb, wkvb_sb),
                             (htb_sb, wkva_sb)]

                    def proj(ti):
                        if ti == 0:
                            box["ps"] = a_psum(tag)
                            box["kvraw"] = sbA.tile([128, 512], BF16,
                                                    name="kvraw", tag="kvraw",
                                                    bufs=3)
                        a_t, w_t = terms[ti]
                        for pr in range(4):
                            nc.tensor.matmul(box["ps"], w_t[:, pr, :, :],
                                             a_t[:, pr, :, csl],
                                             start=(ti == 0 and pr == 0),
                                             stop=(ti == 2 and pr == 3),
                                             perf_mode=DR)
                        if ti == 2:
                            evc(box["kvraw"], box["ps"])

                    def krot():
                        ps_kr = a_psum(tag)[0:64, :]
                        kvraw = box["kvraw"]
                        nc.tensor.matmul(ps_kr, psig_sb[0:64, 0:64],
                                         kvraw[0:64, :], start=True,
                                         stop=True)
                        kdst = kT[0:64, csl]
                        nc.vector.tensor_mul(kdst, kvraw[0:64, :],
                                             cos_sb[0:64, csl])
                        ktmp = sbA.tile([64, 512], BF16, name="ktmp",
                                        tag="ktmp", bufs=2)
                        nc.vector.tensor_mul(ktmp, ps_kr, sin_sb[0:64, csl])
                        nc.vector.tensor_add(kdst, kdst, ktmp)

                    def vt(tt):
                        ti = 4 * sc + tt
                        ps_v = a_psum(tag)
                        ps_vb = ps_v.bitcast(BF16)[:, 0:64]
                        nc.tensor.matmul(
                            ps_vb,
                            box["kvraw"][64:128, 128 * tt:128 * tt + 128],
                            id_sb[64:128, 64:128],
                            start=True, stop=True, is_transpose=True)
                        nc.vector.tensor_scalar_mul(vsm[:, ti, 0:64],
                                                    ps_vb, 1.0 / WS)

                    return ([lambda ti=ti: proj(ti) for ti in range(3)]
                            + [krot]
                            + [lambda tt=tt: vt(tt) for tt in range(4)])

                def steps_q(sc, pp, tag="ps_d", ev=None):
                    evc = (nc.scalar.copy if ev is nc.scalar
                           else nc.vector.tensor_copy)
                    csl = slice(512 * sc, 512 * sc + 512)
                    box = {}

                    terms = [(hta_sb, wqa_sb), (hta_sb, wqb_sb),
                             (htb_sb, wqa_sb)]

                    def proj(ti):
                        if ti == 0:
                            if pp == 1:
                                # deferred K duplicate (waits settled by now)
                                nc.sync.dma_start(out=kT[64:128, csl],
                                                  in_=kT[0:64, csl])
                            box["ps"] = a_psum(tag)
                            box["qraw"] = sbA.tile([128, 512], BF16,
                                                   name="qraw", tag="qraw",
                                                   bufs=3)
                        a_t, w_t = terms[ti]
                        for pr in range(4):
                            nc.tensor.matmul(
                                box["ps"],
                                w_t[:, pr, :, 128 * pp:128 * pp + 128],
                                a_t[:, pr, :, csl],
                                start=(ti == 0 and pr == 0),
                                stop=(ti == 2 and pr == 3),
                                perf_mode=DR)
                        if ti == 2:
                            evc(box["qraw"], box["ps"])

                    def qrot():
                        qraw = box["qraw"]
                        ps_r = a_psum(tag)
                        nc.tensor.matmul(ps_r, psig_sb, qraw, start=True,
                                         stop=True)
                        dst = qTs[pp][:, csl]
                        nc.vector.tensor_mul(dst, qraw, cos_sb[:, csl])
                        rtmp = sbA.tile([128, 512], BF16, name="rtmp",
                                        tag="rtmp", bufs=2)
                        nc.vector.tensor_mul(rtmp, ps_r, sin_sb[:, csl])
                        nc.vector.tensor_add(dst, dst, rtmp)

                    return ([lambda ti=ti: proj(ti) for ti in range(3)]
                            + [qrot])

                # ---------------- Phase BC: attention ----------------
                def emit_bc(qb, filler):
                    """Attention for q block qb, 4 heads; unit-pipelined.

                    filler: list of closures emitting independent PE work,
                    popped between units to cover exp latency.
                    """
                    qsl = slice(512 * qb, 512 * qb + 512)
                    nfull = 4 * qb if causal else NT
                    pending = []
                    n_units = 4 * ((nfull + 1) // 2 + (2 if causal else 0))
                    bstate = {"left": max(n_units, 1), "carry": 0.0}

                    def boundary(flush=False):
                        # deferred emissions first (their waits are settled),
                        # then evenly-paced independent PE filler work
                        for _ in range(len(pending)):
                            pending.pop(0)()
                        if flush:
                            n = len(filler)
                        else:
                            bstate["carry"] += len(filler) / bstate["left"]
                            n = int(bstate["carry"])
                            bstate["carry"] -= n
                            bstate["left"] = max(bstate["left"] - 1, 1)
                        for _ in range(n):
                            if filler:
                                filler.pop(0)()

                    for h in range(4):
                        off = 64 * (h % 2)
                        pp = h // 2
                        ps_qm = psQ.tile([128, 4, 128], F32, name="ps_qm",
                                         tag="ps_qm", bufs=2)
                        # one accumulation group per psum bank: zero the whole
                        # bank once, then every ctx matmul accumulates.  After
                        # the first per-qc stop the bank flag is cleared, so
                        # later accumulates skip the group check.
                        nc.tensor.matmul(ps_qm[:, :, :].opt(), pt_sb, zer_sb,
                                         start=True, stop=False)
                        hstate = {"stopped": False}

                        def ctx_mm(ki, et_ap, qc):
                            # ctx[qc] += et_chunk.T @ (V||1)
                            last = (4 * qb + qc) if causal else (NT - 1)
                            stop = (ki == last)
                            nc.tensor.matmul(
                                ps_qm[:, qc, 0:65], et_ap, vsm[:, ki, :],
                                start=False, stop=stop,
                                skip_group_check=hstate["stopped"])
                            if stop:
                                hstate["stopped"] = True

                        units = []

                        def mk_pair(kp, _off=off, _pp=pp):
                            box = {}

                            def s():
                                ps_s = psS.tile([128, 1024], F32, name="ps_s",
                                                tag="ps_s", bufs=2)
                                for jj in range(2):
                                    ki = kp + jj
                                    nc.tensor.matmul(
                                        ps_s[:, 512 * jj:512 * jj + 512],
                                        kT[_off:_off + 64,
                                           128 * ki:128 * ki + 128],
                                        qTs[_pp][_off:_off + 64, qsl],
                                        start=True, stop=True)
                                box["ps"] = ps_s

                            def ec():
                                et = etp.tile([128, 1024], BF16, name="et",
                                              tag="et", bufs=6)
                                nc.scalar.activation(et, box["ps"], AF.Exp,
                                                     scale=SCALE)
                                for qc in range(4):
                                    for jj in range(2):
                                        ctx_mm(kp + jj,
                                               et[:, 512 * jj + 128 * qc:
                                                  512 * jj + 128 * qc + 128],
                                               qc)
                            return (s, ec)

                        def mk_diagA(_off=off, _pp=pp, _qb=qb):
                            # kv tiles 4qb+0 (span 512) and 4qb+1 (span 384)
                            box = {}
                            k0 = 4 * _qb

                            def s():
                                ps = psS.tile([128, 1024], F32, name="ps_dA",
                                              tag="ps_s", bufs=2)
                                nc.tensor.matmul(
                                    ps[:, 0:512],
                                    kT[_off:_off + 64,
                                       128 * k0:128 * k0 + 128],
                                    qTs[_pp][_off:_off + 64, qsl],
                                    start=True, stop=False)
                                nc.tensor.matmul(
                                    ps[:, 0:128], pt_sb, id_sb,
                                    start=False, stop=True)
                                nc.tensor.matmul(
                                    ps[:, 512:896],
                                    kT[_off:_off + 64,
                                       128 * k0 + 128:128 * k0 + 256],
                                    qTs[_pp][_off:_off + 64,
                                             512 * _qb + 128:512 * _qb + 512],
                                    start=True, stop=False)
                                nc.tensor.matmul(
                                    ps[:, 512:640], pt_sb, id_sb,
                                    start=False, stop=True)
                                box["ps"] = ps

                            def ec():
                                et = etp.tile([128, 896], BF16, name="etdA",
                                              tag="etdA", bufs=3)
                                nc.scalar.activation(et, box["ps"][:, 0:896],
                                                     AF.Exp, scale=SCALE)
                                for qc in range(4):
                                    ctx_mm(k0, et[:, 128 * qc:128 * qc + 128],
                                           qc)
                                for qc in range(1, 4):
                                    ctx_mm(k0 + 1,
                                           et[:, 512 + 128 * (qc - 1):
                                              512 + 128 * qc], qc)
                            return (s, ec)

                        def mk_diagB(_off=off, _pp=pp, _qb=qb):
                            # kv tiles 4qb+2 (span 256) and 4qb+3 (span 128)
                            box = {}
                            k2 = 4 * _qb + 2

                            def s():
                                ps = psS.tile([128, 1024], F32, name="ps_dB",
                                              tag="ps_s", bufs=2)
                                # one group per bank: the [P|0|P] mask
                                # preload writes (and zeroes) every byte the
                                # exp will read, then scores accumulate
                                nc.tensor.matmul(
                                    ps[:, 0:384], pt_sb, m384_sb,
                                    start=True, stop=False)
                                nc.tensor.matmul(
                                    ps[:, 0:256],
                                    kT[_off:_off + 64,
                                       128 * k2:128 * k2 + 128],
                                    qTs[_pp][_off:_off + 64,
                                             512 * _qb + 256:512 * _qb + 512],
                                    start=False, stop=False)
                                nc.tensor.matmul(
                                    ps[:, 256:384],
                                    kT[_off:_off + 64,
                                       128 * k2 + 128:128 * k2 + 256],
                                    qTs[_pp][_off:_off + 64,
                                             512 * _qb + 384:512 * _qb + 512],
                                    start=False, stop=True)
                                box["ps"] = ps

                            def ec():
                                et = etp.tile([128, 384], BF16, name="etdB",
                                              tag="etdB", bufs=3)
                                nc.scalar.activation(et, box["ps"][:, 0:384],
                                                     AF.Exp, scale=SCALE)
                                for qc in range(2, 4):
                                    ctx_mm(k2, et[:, 128 * (qc - 2):
                                                  128 * (qc - 1)], qc)
                                ctx_mm(k2 + 1, et[:, 256:384], 3)
                            return (s, ec)

                        for kp in range(0, nfull, 2):
                            units.append(mk_pair(kp))
                        if causal:
                            units.append(mk_diagA())
                            units.append(mk_diagB())

                        # normalize + evict q-major ctx (bf16); deferred one
                        # unit so the semaphore waits never clog DVE's queue
                        def normalize(_h=h, _ps=ps_qm, _qb=qb, qcs=(0, 1, 2, 3),
                                      tail_h=False):
                            rcp = sbC.tile([128, len(qcs)], F32, name="rcp",
                                           tag="rcp", bufs=4)
                            nc.vector.reciprocal_approx_fast(
                                rcp, _ps[:, qcs[0]:qcs[-1] + 1, 64:65].opt())
                            dstq = state["ctxq"][_h // 2]
                            dstT = state["ctxT"][_h // 2]
                            col = 64 * (_h % 2)
                            for i, qc in enumerate(qcs):
                                nc.vector.tensor_scalar_mul(
                                    dstq[:, qc, col:col + 64],
                                    _ps[:, qc, 0:64], rcp[:, i:i + 1])
                                if tail_h:
                                    # progressive per-qc transpose right
                                    # behind each normalize chunk, on
                                    # alternating queues
                                    eng = nc.sync if qc % 2 == 0 else nc.scalar
                                    eng.dma_start_transpose(
                                        dstT[:, qc, :], dstq[:, qc, :])
                            if _h % 2 == 1 and not tail_h:
                                def transpose(_h2=_h // 2):
                                    dT = state["ctxT"][_h2]
                                    dq = state["ctxq"][_h2]
                                    nc.sync.dma_start_transpose(
                                        dT[:, :, :], dq[:, :, :])
                                pending.append(transpose)

                        is_tail_h = (causal and qb == NQB - 1 and h == 3)
                        if units:
                            units[0][0]()
                        for i in range(len(units)):
                            if i + 1 < len(units):
                                units[i + 1][0]()
                            boundary()
                            units[i][1]()
                            if is_tail_h and i == len(units) - 2:
                                # qc0/1 groups stopped in diagA: normalize
                                # them while diagB is still in flight
                                pending.append(
                                    lambda: normalize(qcs=(0, 1),
                                                      tail_h=True))

                        if is_tail_h:
                            pending.append(lambda: normalize(qcs=(2, 3),
                                                             tail_h=True))
                        else:
                            pending.append(normalize)
                    while pending:
                        pending.pop(0)()
                    boundary(flush=True)

                # ---------------- Phase D: output projection ----------------
                def emit_d(qb, ctxTA, ctxTB, tail=False):
                    out = []
                    for qt in range(4):
                        for nb in range(2):
                            def f(_qt=qt, _nb=nb, _qb=qb, _A=ctxTA, _B=ctxTB):
                                ps_o = psD.tile([128, 512], F32, name="ps_o",
                                                tag="ps_d", bufs=2)
                                nsl = slice(512 * _nb, 512 * _nb + 512)
                                nc.tensor.matmul(ps_o, _A[:, _qt, :],
                                                 wo_sb[:, 0, nsl],
                                                 start=True, stop=False)
                                nc.tensor.matmul(ps_o, _B[:, _qt, :],
                                                 wo_sb[:, 1, nsl],
                                                 start=False, stop=True)
                                ost = sbC.tile([128, 512], BF16, name="ost",
                                               tag="ost", bufs=16)
                                if tail and _nb == 1:
                                    nc.scalar.copy(ost, ps_o)
                                else:
                                    nc.vector.tensor_copy(ost, ps_o)
                                dst = outp[512 * _qb + 128 * _qt:
                                           512 * _qb + 128 * _qt + 128, nsl]
                                if tail:
                                    nc.sync.dma_start(out=dst, in_=ost)
                                else:
                                    # deferred so its wait is settled by the
                                    # time it hits the SP queue
                                    state["outq"].append((dst, ost))
                            out.append(f)
                    return out

                # ---------------- global schedule ----------------
                def afiller(sc):
                    return (steps_kv(sc) + steps_q(sc, 0) + steps_q(sc, 1))

                def rotate_ctx():
                    state["ctxq"] = [
                        sbC.tile([128, 4, 128], BF16, name=f"ctxq{h2}",
                                 tag=f"ctxq{h2}", bufs=2) for h2 in range(2)]
                    state["ctxT"] = [
                        sbC.tile([128, 4, 128], BF16, name=f"ctxT{h2}",
                                 tag=f"ctxT{h2}", bufs=2) for h2 in range(2)]

                if causal:
                    # wavefront: BC(qb) only needs kv/q chunks <= qb.
                    # Prologue: kv proj -> q proj (borrowing score banks) ->
                    # k rope -> q rope; V transposes land in BC(0)'s filler.
                    kv0 = steps_kv(0)
                    q00 = steps_q(0, 0, tag="ps_s")
                    for st in kv0[0:3]:         # kv proj
                        st()
                    for st in q00[0:3]:         # q proj (ps_s banks)
                        st()
                    kv0[3]()                    # k rope
                    q00[3]()                    # q rope
                    emit_bc(0, kv0[4:8] + steps_q(0, 1) + afiller(1))
                    for qb in range(1, NQB):
                        dA = emit_d(qb - 1, state["ctxT"][0],
                                    state["ctxT"][1])
                        rotate_ctx()
                        fill = (afiller(qb + 1) if qb + 1 < NQB else [])
                        fill += [flush_outq] + dA
                        emit_bc(qb, fill)
                else:
                    # full attention needs all kv before any BC
                    for sc in range(4):
                        for st in steps_kv(sc) + steps_q(sc, 0) \
                                + steps_q(sc, 1):
                            st()
                    emit_bc(0, [])
                    for qb in range(1, NQB):
                        dA = emit_d(qb - 1, state["ctxT"][0],
                                    state["ctxT"][1])
                        rotate_ctx()
                        emit_bc(qb, [flush_outq] + dA)
                for f in emit_d(NQB - 1, state["ctxT"][0], state["ctxT"][1],
                                tail=causal):
                    f()
                flush_outq()

    nc.compile()
    return nc


_NC_CACHE = {}


def _get_nc(causal: bool):
    if causal not in _NC_CACHE:
        _NC_CACHE[causal] = _build_nc(causal)
    return _NC_CACHE[causal]


def _host_consts():
    p = np.zeros((128, 128), np.float32)
    idx = np.arange(0, 128, 2)
    p[idx, idx + 1] = -1.0
    p[idx + 1, idx] = 1.0
    psigT = np.ascontiguousarray(p.T).astype(BF)
    pm = np.where(np.arange(128)[None, :] < np.arange(128)[:, None],
                  np.float32(NEG), np.float32(0.0))
    ptneg = np.ascontiguousarray(pm.T).astype(BF)
    ident = np.eye(128, dtype=np.float32).astype(BF)
    m384 = np.zeros((128, 384), np.float32)
    m384[:, 0:128] = np.eye(128)
    m384[:, 256:384] = np.eye(128)
    return psigT, ptneg, ident, m384.astype(BF)


def _numpy_reference(hidden_states, cos, sin, attention_mask, Wq, Wk, Wv, Wo):
    """Generic-mask fallback, pure numpy port of the reference."""
    GROUPS = H // KVH

    def rope(x, c, s):
        c = c[:, None, :, :]
        s = s[:, None, :, :]
        x1, x2 = x[..., ::2], x[..., 1::2]
        xr = np.stack([x1 * c - x2 * s, x1 * s + x2 * c], axis=-1)
        return xr.reshape(x.shape)

    b, sq, d = hidden_states.shape
    q = (hidden_states @ Wq).reshape(b, sq, H, HD).transpose(0, 2, 1, 3)
    k = (hidden_states @ Wk).reshape(b, sq, KVH, HD).transpose(0, 2, 1, 3)
    v = (hidden_states @ Wv).reshape(b, sq, KVH, HD).transpose(0, 2, 1, 3)
    q = rope(q, cos, sin)
    k = rope(k, cos, sin)
    k = np.repeat(k, GROUPS, axis=1)
    v = np.repeat(v, GROUPS, axis=1)
    out = np.zeros((b, sq, d), np.float32)
    for bi in range(b):
        for hi in range(H):
            sc = (q[bi, hi] @ k[bi, hi].T) * SCALE + attention_mask[0, 0]
            sc = sc - sc.max(axis=-1, keepdims=True)
            e = np.exp(sc)
            pr = e / e.sum(axis=-1, keepdims=True)
            ctx = pr @ v[bi, hi]
            out[bi] += ctx @ Wo[hi * HD:(hi + 1) * HD]
    return out


def kernel(**inputs) -> np.ndarray:
    hs = np.asarray(inputs["hidden_states"], np.float32)
    cos = np.asarray(inputs["cos"], np.float32)
    sin = np.asarray(inputs["sin"], np.float32)
    mask = np.asarray(inputs["attention_mask"], np.float32)
    Wq = np.asarray(inputs["Wq"], np.float32)
    Wk = np.asarray(inputs["Wk"], np.float32)
    Wv = np.asarray(inputs["Wv"], np.float32)
    Wo = np.asarray(inputs["Wo"], np.float32)

    m = mask.reshape(S, S)
    tril = np.tril(np.ones((S, S), dtype=bool))
    causal_ref = np.where(tril, np.float32(0.0), np.float32(NEG))
    if np.array_equal(m, causal_ref):
        causal = True
    elif not m.any():
        causal = False
    else:
        return _numpy_reference(hs, cos, sin, mask, Wq, Wk, Wv, Wo)

    nc = _get_nc(causal)
    psigT, ptneg, ident, m384 = _host_consts()
    chan_half = (np.arange(64) // 2)

    in_maps = []
    for core in range(8):
        b, t = core // TP, core % TP
        hTf = np.ascontiguousarray(hs[b].T)                       # [D, S]
        h1, h2 = _split8(hTf)
        ht8a = _pairs(h1, S)
        ht8b = _pairs(h2, S)
        cs64 = np.ascontiguousarray(cos[b].T[chan_half, :])       # [64, S]
        sn64 = np.ascontiguousarray(sin[b].T[chan_half, :])
        csd = (np.vstack([cs64, cs64]) / WS).astype(BF)
        snd = (np.vstack([sn64, sn64]) / WS).astype(BF)
        wq1, wq2 = _split8(Wq[:, t * 256:(t + 1) * 256] * WS)
        wkv_f = np.concatenate([Wk[:, t * 64:(t + 1) * 64],
                                Wv[:, t * 64:(t + 1) * 64]], axis=1)
        wk1, wk2 = _split8(wkv_f * WS)
        wo_s = np.ascontiguousarray(
            Wo[t * 256:(t + 1) * 256].reshape(2, 128, D)
            .transpose(1, 0, 2)).astype(BF)
        in_maps.append({
            "ht8a": ht8a, "ht8b": ht8b, "csd": csd, "snd": snd,
            "wq8a": _pairs(wq1, 256), "wq8b": _pairs(wq2, 256),
            "wkv8a": _pairs(wk1, 128), "wkv8b": _pairs(wk2, 128),
            "wo": wo_s,
            "psigT": psigT, "ptneg": ptneg, "ident": ident, "m384": m384,
        })

    res = run_bass_kernel_spmd(nc, in_maps, core_ids=list(range(8)))
    out = np.zeros((B, S, D), np.float32)
    for core in range(8):
        out[core // TP] += res.results[core]["out"].astype(np.float32)
    return out


# revision 59
# speedup vs baseline: 1.3666x; 1.0054x over previous
"""Self-contained Trainium2 Bass kernel for GQA MultiHeadAttention with RoPE.

Problem: B=2, S=2048, D=1024, H=16 Q heads, KVH=4 KV heads, head_dim=64,
causal additive mask, f32.

Sharding: tensor-parallel over heads (TP=4: 4 Q heads + 1 KV head per shard)
x data-parallel over batch (DP=2) = 8 NeuronCores. Wo is sharded on its
input dim; the host sums the 4 partial outputs per batch element.

Design notes (all bf16 datapath, f32 psum):
 - scores kept kv-major ([kv, q] psum tiles) so exp feeds strictly from PE;
   causal diagonal handled by a -1e9 mask ADDED via a PT @ I matmul into the
   same psum accumulation group (no post-exp mask multiplies).
 - ctx computed q-major: stationary = exp'd scores chunk [kv,128q], moving =
   V||ones [kv,65]  ->  psum [128q, 65].  The softmax denominator lands on
   column 64, per-partition, so normalize = reciprocal + tensor_scalar_mul,
   no DRAM round trip.
 - ctx transposed back to ch-major for the output projection with the
   DMA xbar transpose (SBUF->SBUF, bf16).
 - Activation engine runs exps only; evictions go to DVE/ACT split; GPSIMD
   cannot touch PSUM.
"""

import os
import sys

for _p in ("/opt/trn_rl_repo", "/root/.axon_site/_ro/trn_rl_repo"):
    if os.path.isdir(_p) and _p not in sys.path:
        sys.path.insert(0, _p)

import numpy as np
import ml_dtypes

import concourse.bacc as bacc
import concourse.bass as bass
import concourse.tile as tile
from concourse import mybir
from concourse.bass_utils import run_bass_kernel_spmd

F32 = mybir.dt.float32
BF16 = mybir.dt.bfloat16
F8 = mybir.dt.float8e4
DR = mybir.MatmulPerfMode.DoubleRow
WS = 16.0                   # fp8 weight pre-scale (split residual headroom)
AF = mybir.ActivationFunctionType
BF = ml_dtypes.bfloat16
F8NP = ml_dtypes.float8_e4m3fn


def _split8(x):
    """fp8 value/residual split (f32 -> two e4m3 arrays)."""
    x1 = x.astype(F8NP)
    x2 = (x - x1.astype(np.float32)).astype(F8NP)
    return x1, x2


def _pairs(x, ncols):
    """[D, N] -> [128, 4, 2, N] DoubleRow pair layout."""
    return np.ascontiguousarray(
        x.reshape(4, 2, 128, ncols).transpose(2, 0, 1, 3))

H, KVH, HD = 16, 4, 64
B, S, D = 2, 2048, 1024
TP = 4                      # head-parallel ways
SCALE = HD ** -0.5
NEG = -1e9
NT = S // 128               # 16 kv tiles
NQB = S // 512              # 4 q blocks


def _patch_act_tables():
    """Make Exp resolve only to natural_log_exp_and_others so the act-table
    pass emits a single table load."""
    from concourse.hw_specs import get_activation_tables
    t = get_activation_tables("gen3")
    for name, fns in t.items():
        if name != "natural_log_exp_and_others":
            fns.discard(AF.Exp)
            fns.discard(AF.Ln)


def _build_nc(causal: bool):
    _patch_act_tables()
    nc = bacc.Bacc()

    ht8a = nc.declare_dram_parameter("ht8a", [128, 4, 2, S], F8, isOutput=False)
    ht8b = nc.declare_dram_parameter("ht8b", [128, 4, 2, S], F8, isOutput=False)
    csd = nc.declare_dram_parameter("csd", [128, S], BF16, isOutput=False)
    snd = nc.declare_dram_parameter("snd", [128, S], BF16, isOutput=False)
    wq8a = nc.declare_dram_parameter("wq8a", [128, 4, 2, 256], F8, isOutput=False)
    wq8b = nc.declare_dram_parameter("wq8b", [128, 4, 2, 256], F8, isOutput=False)
    wkv8a = nc.declare_dram_parameter("wkv8a", [128, 4, 2, 128], F8, isOutput=False)
    wkv8b = nc.declare_dram_parameter("wkv8b", [128, 4, 2, 128], F8, isOutput=False)
    wo = nc.declare_dram_parameter("wo", [128, 2, D], BF16, isOutput=False)
    psigT = nc.declare_dram_parameter("psigT", [128, 128], BF16, isOutput=False)
    ptneg = nc.declare_dram_parameter("ptneg", [128, 128], BF16, isOutput=False)
    ident = nc.declare_dram_parameter("ident", [128, 128], BF16, isOutput=False)
    m384 = nc.declare_dram_parameter("m384", [128, 384], BF16, isOutput=False)
    outp = nc.declare_dram_parameter("out", [S, D], BF16, isOutput=True)

    with tile.TileContext(nc) as tc:
        with tc.tile_pool(name="hold", bufs=1) as hp:
            # ---- constants / weights (two DMA queues) ----
            # single sync queue, strict priority order for the first-exp path
            wkva_sb = hp.tile([128, 4, 2, 128], F8, name="wkva_sb",
                              tag="wkva_sb")
            nc.sync.dma_start(out=wkva_sb, in_=wkv8a[:, :, :, :])
            hta_sb = hp.tile([128, 4, 2, S], F8, name="hta_sb", tag="hta_sb")
            htb_sb = hp.tile([128, 4, 2, S], F8, name="htb_sb", tag="htb_sb")
            nc.sync.dma_start(out=hta_sb[:, :, :, 0:512],
                              in_=ht8a[:, :, :, 0:512])
            wkvb_sb = hp.tile([128, 4, 2, 128], F8, name="wkvb_sb",
                              tag="wkvb_sb")
            nc.sync.dma_start(out=wkvb_sb, in_=wkv8b[:, :, :, :])
            nc.sync.dma_start(out=htb_sb[:, :, :, 0:512],
                              in_=ht8b[:, :, :, 0:512])
            cos_sb = hp.tile([128, S], BF16, name="cos_sb", tag="cos_sb")
            sin_sb = hp.tile([128, S], BF16, name="sin_sb", tag="sin_sb")
            nc.sync.dma_start(out=cos_sb[:, 0:512], in_=csd[:, 0:512])
            nc.sync.dma_start(out=sin_sb[:, 0:512], in_=snd[:, 0:512])
            psig_sb = hp.tile([128, 128], BF16, name="psig_sb", tag="psig_sb")
            nc.sync.dma_start(out=psig_sb, in_=psigT[:, :])
            pt_sb = hp.tile([128, 128], BF16, name="pt_sb", tag="pt_sb")
            nc.sync.dma_start(out=pt_sb, in_=ptneg[:, :])
            id_sb = hp.tile([128, 128], BF16, name="id_sb", tag="id_sb")
            nc.sync.dma_start(out=id_sb, in_=ident[:, :])
            m384_sb = hp.tile([128, 384], BF16, name="m384_sb", tag="m384_sb")
            nc.sync.dma_start(out=m384_sb, in_=m384[:, :])
            wqa_sb = hp.tile([128, 4, 2, 256], F8, name="wqa_sb", tag="wqa_sb")
            nc.sync.dma_start(out=wqa_sb, in_=wq8a[:, :, :, :])
            wqb_sb = hp.tile([128, 4, 2, 256], F8, name="wqb_sb", tag="wqb_sb")
            nc.sync.dma_start(out=wqb_sb, in_=wq8b[:, :, :, :])
            nc.sync.dma_start(out=hta_sb[:, :, :, 512:1024],
                              in_=ht8a[:, :, :, 512:1024])
            nc.sync.dma_start(out=htb_sb[:, :, :, 512:1024],
                              in_=ht8b[:, :, :, 512:1024])
            nc.sync.dma_start(out=cos_sb[:, 512:S], in_=csd[:, 512:S])
            nc.sync.dma_start(out=sin_sb[:, 512:S], in_=snd[:, 512:S])
            for sc in range(2, 4):
                csl = slice(512 * sc, 512 * sc + 512)
                nc.sync.dma_start(out=hta_sb[:, :, :, csl],
                                  in_=ht8a[:, :, :, csl])
                nc.sync.dma_start(out=htb_sb[:, :, :, csl],
                                  in_=ht8b[:, :, :, csl])
            wo_sb = hp.tile([128, 2, D], BF16, name="wo_sb", tag="wo_sb")
            nc.sync.dma_start(out=wo_sb, in_=wo[:, :, :])

            qTs = [hp.tile([128, S], BF16, name=f"qT{p}", tag=f"qT{p}")
                   for p in range(2)]
            kT = hp.tile([128, S], BF16, name="kTt", tag="kTt")
            vsm = hp.tile([128, NT, 65], BF16, name="vsm", tag="vsm")
            nc.gpsimd.memset(vsm[:, :, 64:65], 1.0)
            zer_sb = hp.tile([128, 512], BF16, name="zer_sb", tag="zer_sb")
            nc.gpsimd.memset(zer_sb, 0.0)

            with tc.tile_pool(name="psS", bufs=1, space="PSUM") as psS, \
                 tc.tile_pool(name="psD", bufs=1, space="PSUM") as psD, \
                 tc.tile_pool(name="psQ", bufs=1, space="PSUM") as psQ, \
                 tc.tile_pool(name="etp", bufs=1) as etp, \
                 tc.tile_pool(name="sbA", bufs=1) as sbA, \
                 tc.tile_pool(name="sbC", bufs=1) as sbC:

                # per-qb rotating ctx tiles (q-major and transposed ch-major)
                # A: heads 0,1 (ch 0:128); B: heads 2,3 (ch 128:256)
                ctxq = [sbC.tile([128, 4, 128], BF16, name=f"ctxq{h2}",
                                 tag=f"ctxq{h2}", bufs=2) for h2 in range(2)]
                ctxT = [sbC.tile([128, 4, 128], BF16, name=f"ctxT{h2}",
                                 tag=f"ctxT{h2}", bufs=2) for h2 in range(2)]

                state = {"ctxq": [ctxq[0], ctxq[1]],
                         "ctxT": [ctxT[0], ctxT[1]],
                         "outq": []}

                def flush_outq():
                    for dst, ost in state["outq"]:
                        nc.sync.dma_start(out=dst, in_=ost)
                    state["outq"] = []

                # ---------------- Phase A: projections + rope ----------------
                # Emitted as ~0.5us micro-steps so interleaving into the BC
                # unit stream never starves the exp cadence.
                def a_psum(tag):
                    # prologue can borrow the (idle) score banks
                    if tag == "ps_s":
                        return psS.tile([128, 1024], F32, name="ps_a",
                                        tag="ps_s", bufs=2)[:, 0:512]
                    return psD.tile([128, 512], F32, name="ps_a",
                                    tag="ps_d", bufs=2)

                def steps_kv(sc, tag="ps_d", ev=None):
                    evc = (nc.scalar.copy if ev is nc.scalar
                           else nc.vector.tensor_copy)
                    csl = slice(512 * sc, 512 * sc + 512)
                    box = {}

                    # 3-term fp8 DoubleRow split: a1*w1, a1*w2, a2*w1
                    terms = [(hta_sb, wkva_sb), (hta_sb, wkvb_sb),
                             (htb_sb, wkva_sb)]

                    def proj(ti):
                        if ti == 0:
                            box["ps"] = a_psum(tag)
                            box["kvraw"] = sbA.tile([128, 512], BF16,
                                                    name="kvraw", tag="kvraw",
                                                    bufs=3)
                        a_t, w_t = terms[ti]
                        for pr in range(4):
                            nc.tensor.matmul(box["ps"], w_t[:, pr, :, :],
                                             a_t[:, pr, :, csl],
                                             start=(ti == 0 and pr == 0),
                                             stop=(ti == 2 and pr == 3),
                                             perf_mode=DR)
                        if ti == 2:
                            evc(box["kvraw"], box["ps"])

                    def krot():
                        ps_kr = a_psum(tag)[0:64, :]
                        kvraw = box["kvraw"]
                        nc.tensor.matmul(ps_kr, psig_sb[0:64, 0:64],
                                         kvraw[0:64, :], start=True,
                                         stop=True)
                        kdst = kT[0:64, csl]
                        nc.vector.tensor_mul(kdst, kvraw[0:64, :],
                                             cos_sb[0:64, csl])
                        ktmp = sbA.tile([64, 512], BF16, name="ktmp",
                                        tag="ktmp", bufs=2)
                        nc.vector.tensor_mul(ktmp, ps_kr, sin_sb[0:64, csl])
                        nc.vector.tensor_add(kdst, kdst, ktmp)

                    def vt(tt):
                        ti = 4 * sc + tt
                        ps_v = a_psum(tag)
                        ps_vb = ps_v.bitcast(BF16)[:, 0:64]
                        nc.tensor.matmul(
                            ps_vb,
                            box["kvraw"][64:128, 128 * tt:128 * tt + 128],
                            id_sb[64:128, 64:128],
                            start=True, stop=True, is_transpose=True)
                        nc.vector.tensor_scalar_mul(vsm[:, ti, 0:64],
                                                    ps_vb, 1.0 / WS)

                    return ([lambda ti=ti: proj(ti) for ti in range(3)]
                            + [krot]
                            + [lambda tt=tt: vt(tt) for tt in range(4)])

                def steps_q(sc, pp, tag="ps_d", ev=None):
                    evc = (nc.scalar.copy if ev is nc.scalar
                           else nc.vector.tensor_copy)
                    csl = slice(512 * sc, 512 * sc + 512)
                    box = {}

                    terms = [(hta_sb, wqa_sb), (hta_sb, wqb_sb),
                             (htb_sb, wqa_sb)]

                    def proj(ti):
                        if ti == 0:
                            if pp == 1:
                                # deferred K duplicate (waits settled by now)
                                nc.sync.dma_start(out=kT[64:128, csl],
                                                  in_=kT[0:64, csl])
                            box["ps"] = a_psum(tag)
                            box["qraw"] = sbA.tile([128, 512], BF16,
                                                   name="qraw", tag="qraw",
                                                   bufs=3)
                        a_t, w_t = terms[ti]
                        for pr in range(4):
                            nc.tensor.matmul(
                                box["ps"],
                                w_t[:, pr, :, 128 * pp:128 * pp + 128],
                                a_t[:, pr, :, csl],
                                start=(ti == 0 and pr == 0),
                                stop=(ti == 2 and pr == 3),
                                perf_mode=DR)
                        if ti == 2:
                            evc(box["qraw"], box["ps"])

                    def qrot():
                        qraw = box["qraw"]
                        ps_r = a_psum(tag)
                        nc.tensor.matmul(ps_r, psig_sb, qraw, start=True,
                                         stop=True)
                        dst = qTs[pp][:, csl]
                        nc.vector.tensor_mul(dst, qraw, cos_sb[:, csl])
                        rtmp = sbA.tile([128, 512], BF16, name="rtmp",
                                        tag="rtmp", bufs=2)
                        nc.vector.tensor_mul(rtmp, ps_r, sin_sb[:, csl])
                        nc.vector.tensor_add(dst, dst, rtmp)

                    return ([lambda ti=ti: proj(ti) for ti in range(3)]
                            + [qrot])

                # ---------------- Phase BC: attention ----------------
                def emit_bc(qb, filler):
                    """Attention for q block qb, 4 heads; unit-pipelined.

                    filler: list of closures emitting independent PE work,
                    popped between units to cover exp latency.
                    """
                    qsl = slice(512 * qb, 512 * qb + 512)
                    nfull = 4 * qb if causal else NT
                    pending = []
                    n_units = 4 * ((nfull + 1) // 2 + (2 if causal else 0))
                    bstate = {"left": max(n_units, 1), "carry": 0.0}

                    def boundary(flush=False):
                        # deferred emissions first (their waits are settled),
                        # then evenly-paced independent PE filler work
                        for _ in range(len(pending)):
                            pending.pop(0)()
                        if flush:
                            n = len(filler)
                        else:
                            bstate["carry"] += len(filler) / bstate["left"]
                            n = int(bstate["carry"])
                            bstate["carry"] -= n
                            bstate["left"] = max(bstate["left"] - 1, 1)
                        for _ in range(n):
                            if filler:
                                filler.pop(0)()

                    for h in range(4):
                        off = 64 * (h % 2)
                        pp = h // 2
                        ps_qm = psQ.tile([128, 4, 128], F32, name="ps_qm",
                                         tag="ps_qm", bufs=2)
                        # one accumulation group per psum bank: zero the whole
                        # bank once, then every ctx matmul accumulates.  After
                        # the first per-qc stop the bank flag is cleared, so
                        # later accumulates skip the group check.
                        nc.tensor.matmul(ps_qm[:, :, :].opt(), pt_sb, zer_sb,
                                         start=True, stop=False)
                        hstate = {"stopped": False}

                        def ctx_mm(ki, et_ap, qc):
                            # ctx[qc] += et_chunk.T @ (V||1)
                            last = (4 * qb + qc) if causal else (NT - 1)
                            stop = (ki == last)
                            nc.tensor.matmul(
                                ps_qm[:, qc, 0:65], et_ap, vsm[:, ki, :],
                                start=False, stop=stop,
                                skip_group_check=hstate["stopped"])
                            if stop:
                                hstate["stopped"] = True

                        units = []

                        def mk_pair(kp, _off=off, _pp=pp):
                            box = {}

                            def s():
                                ps_s = psS.tile([128, 1024], F32, name="ps_s",
                                                tag="ps_s", bufs=2)
                                for jj in range(2):
                                    ki = kp + jj
                                    nc.tensor.matmul(
                                        ps_s[:, 512 * jj:512 * jj + 512],
                                        kT[_off:_off + 64,
                                           128 * ki:128 * ki + 128],
                                        qTs[_pp][_off:_off + 64, qsl],
                                        start=True, stop=True)
                                box["ps"] = ps_s

                            def ec():
                                et = etp.tile([128, 1024], BF16, name="et",
                                              tag="et", bufs=6)
                                nc.scalar.activation(et, box["ps"], AF.Exp,
                                                     scale=SCALE)
                                for qc in range(4):
                                    for jj in range(2):
                                        ctx_mm(kp + jj,
                                               et[:, 512 * jj + 128 * qc:
                                                  512 * jj + 128 * qc + 128],
                                               qc)
                            return (s, ec)

                        def mk_diagA(_off=off, _pp=pp, _qb=qb):
                            # kv tiles 4qb+0 (span 512) and 4qb+1 (span 384)
                            box = {}
                            k0 = 4 * _qb

                            def s():
                                ps = psS.tile([128, 1024], F32, name="ps_dA",
                                              tag="ps_s", bufs=2)
                                nc.tensor.matmul(
                                    ps[:, 0:512],
                                    kT[_off:_off + 64,
                                       128 * k0:128 * k0 + 128],
                                    qTs[_pp][_off:_off + 64, qsl],
                                    start=True, stop=False)
                                nc.tensor.matmul(
                                    ps[:, 0:128], pt_sb, id_sb,
                                    start=False, stop=True)
                                nc.tensor.matmul(
                                    ps[:, 512:896],
                                    kT[_off:_off + 64,
                                       128 * k0 + 128:128 * k0 + 256],
                                    qTs[_pp][_off:_off + 64,
                                             512 * _qb + 128:512 * _qb + 512],
                                    start=True, stop=False)
                                nc.tensor.matmul(
                                    ps[:, 512:640], pt_sb, id_sb,
                                    start=False, stop=True)
                                box["ps"] = ps

                            def ec():
                                et = etp.tile([128, 896], BF16, name="etdA",
                                              tag="etdA", bufs=3)
                                nc.scalar.activation(et, box["ps"][:, 0:896],
                                                     AF.Exp, scale=SCALE)
                                for qc in range(4):
                                    ctx_mm(k0, et[:, 128 * qc:128 * qc + 128],
                                           qc)
                                for qc in range(1, 4):
                                    ctx_mm(k0 + 1,
                                           et[:, 512 + 128 * (qc - 1):
                                              512 + 128 * qc], qc)
                            return (s, ec)

                        def mk_diagB(_off=off, _pp=pp, _qb=qb):
                            # kv tiles 4qb+2 (span 256) and 4qb+3 (span 128)
                            box = {}
                            k2 = 4 * _qb + 2

                            def s():
                                ps = psS.tile([128, 1024], F32, name="ps_dB",
                                              tag="ps_s", bufs=2)
                                # one group per bank: the [P|0|P] mask
                                # preload writes (and zeroes) every byte the
                                # exp will read, then scores accumulate
                                nc.tensor.matmul(
                                    ps[:, 0:384], pt_sb, m384_sb,
                                    start=True, stop=False)
                                nc.tensor.matmul(
                                    ps[:, 0:256],
                                    kT[_off:_off + 64,
                                       128 * k2:128 * k2 + 128],
                                    qTs[_pp][_off:_off + 64,
                                             512 * _qb + 256:512 * _qb + 512],
                                    start=False, stop=False)
                                nc.tensor.matmul(
                                    ps[:, 256:384],
                                    kT[_off:_off + 64,
                                       128 * k2 + 128:128 * k2 + 256],
                                    qTs[_pp][_off:_off + 64,
                                             512 * _qb + 384:512 * _qb + 512],
                                    start=False, stop=True)
                                box["ps"] = ps

                            def ec():
                                et = etp.tile([128, 384], BF16, name="etdB",
                                              tag="etdB", bufs=3)
                                nc.scalar.activation(et, box["ps"][:, 0:384],
                                                     AF.Exp, scale=SCALE)
                                for qc in range(2, 4):
                                    ctx_mm(k2, et[:, 128 * (qc - 2):
                                                  128 * (qc - 1)], qc)
                                ctx_mm(k2 + 1, et[:, 256:384], 3)
                            return (s, ec)

                        for kp in range(0, nfull, 2):
                            units.append(mk_pair(kp))
                        if causal:
                            units.append(mk_diagA())
                            units.append(mk_diagB())

                        # normalize + evict q-major ctx (bf16); deferred one
                        # unit so the semaphore waits never clog DVE's queue
                        def normalize(_h=h, _ps=ps_qm, _qb=qb, qcs=(0, 1, 2, 3),
                                      tail_h=False):
                            rcp = sbC.tile([128, len(qcs)], F32, name="rcp",
                                           tag="rcp", bufs=4)
                            nc.vector.reciprocal_approx_fast(
                                rcp, _ps[:, qcs[0]:qcs[-1] + 1, 64:65].opt())
                            dstq = state["ctxq"][_h // 2]
                            dstT = state["ctxT"][_h // 2]
                            col = 64 * (_h % 2)
                            for i, qc in enumerate(qcs):
                                nc.vector.tensor_scalar_mul(
                                    dstq[:, qc, col:col + 64],
                                    _ps[:, qc, 0:64], rcp[:, i:i + 1])
                                if tail_h:
                                    # progressive per-qc transpose right
                                    # behind each normalize chunk, on
                                    # alternating queues
                                    eng = nc.sync if qc % 2 == 0 else nc.scalar
                                    eng.dma_start_transpose(
                                        dstT[:, qc, :], dstq[:, qc, :])
                            if _h % 2 == 1 and not tail_h:
                                def transpose(_h2=_h // 2):
                                    dT = state["ctxT"][_h2]
                                    dq = state["ctxq"][_h2]
                                    nc.sync.dma_start_transpose(
                                        dT[:, :, :], dq[:, :, :])
                                pending.append(transpose)

                        is_tail_h = (causal and qb == NQB - 1 and h == 3)
                        if units:
                            units[0][0]()
                        for i in range(len(units)):
                            if i + 1 < len(units):
                                units[i + 1][0]()
                            boundary()
                            units[i][1]()
                            if is_tail_h and i == len(units) - 2:
                                # qc0/1 groups stopped in diagA: normalize
                                # them while diagB is still in flight
                                pending.append(
                                    lambda: normalize(qcs=(0, 1),
                                                      tail_h=True))

                        if is_tail_h:
                            pending.append(lambda: normalize(qcs=(2, 3),
                                                             tail_h=True))
                        else:
                            pending.append(normalize)
                    while pending:
                        pending.pop(0)()
                    boundary(flush=True)

                # ---------------- Phase D: output projection ----------------
                def emit_d(qb, ctxTA, ctxTB, tail=False):
                    out = []
                    for qt in range(4):
                        for nb in range(2):
                            def f(_qt=qt, _nb=nb, _qb=qb, _A=ctxTA, _B=ctxTB):
                                ps_o = psD.tile([128, 512], F32, name="ps_o",
                                                tag="ps_d", bufs=2)
                                nsl = slice(512 * _nb, 512 * _nb + 512)
                                nc.tensor.matmul(ps_o, _A[:, _qt, :],
                                                 wo_sb[:, 0, nsl],
                                                 start=True, stop=False)
                                nc.tensor.matmul(ps_o, _B[:, _qt, :],
                                                 wo_sb[:, 1, nsl],
                                                 start=False, stop=True)
                                ost = sbC.tile([128, 512], BF16, name="ost",
                                               tag="ost", bufs=16)
                                if tail and _nb == 1:
                                    nc.scalar.copy(ost, ps_o)
                                else:
                                    nc.vector.tensor_copy(ost, ps_o)
                                dst = outp[512 * _qb + 128 * _qt:
                                           512 * _qb + 128 * _qt + 128, nsl]
                                if tail:
                                    qeng = [nc.sync, nc.gpsimd,
                                            nc.scalar][(2 * _qt + _nb) % 3]
                                    qeng.dma_start(out=dst, in_=ost)
                                else:
                                    # deferred so its wait is settled by the
                                    # time it hits the SP queue
                                    state["outq"].append((dst, ost))
                            out.append(f)
                    return out

                # ---------------- global schedule ----------------
                def afiller(sc):
                    return (steps_kv(sc) + steps_q(sc, 0) + steps_q(sc, 1))

                def rotate_ctx():
                    state["ctxq"] = [
                        sbC.tile([128, 4, 128], BF16, name=f"ctxq{h2}",
                                 tag=f"ctxq{h2}", bufs=2) for h2 in range(2)]
                    state["ctxT"] = [
                        sbC.tile([128, 4, 128], BF16, name=f"ctxT{h2}",
                                 tag=f"ctxT{h2}", bufs=2) for h2 in range(2)]

                if causal:
                    # wavefront: BC(qb) only needs kv/q chunks <= qb.
                    # Prologue: kv proj -> q proj (borrowing score banks) ->
                    # k rope -> q rope; V transposes land in BC(0)'s filler.
                    kv0 = steps_kv(0)
                    q00 = steps_q(0, 0, tag="ps_s")
                    for st in kv0[0:3]:         # kv proj
                        st()
                    for st in q00[0:3]:         # q proj (ps_s banks)
                        st()
                    kv0[3]()                    # k rope
                    q00[3]()                    # q rope
                    emit_bc(0, kv0[4:8] + steps_q(0, 1) + afiller(1))
                    for qb in range(1, NQB):
                        dA = emit_d(qb - 1, state["ctxT"][0],
                                    state["ctxT"][1])
                        rotate_ctx()
                        fill = (afiller(qb + 1) if qb + 1 < NQB else [])
                        fill += [flush_outq] + dA
                        emit_bc(qb, fill)
                else:
                    # full attention needs all kv before any BC
                    for sc in range(4):
                        for st in steps_kv(sc) + steps_q(sc, 0) \
                                + steps_q(sc, 1):
                            st()
                    emit_bc(0, [])
                    for qb in range(1, NQB):
                        dA = emit_d(qb - 1, state["ctxT"][0],
                                    state["ctxT"][1])
                        rotate_ctx()
                        emit_bc(qb, [flush_outq] + dA)
                for f in emit_d(NQB - 1, state["ctxT"][0], state["ctxT"][1],
                                tail=causal):
                    f()
                flush_outq()

    nc.compile()
    return nc


_NC_CACHE = {}


def _get_nc(causal: bool):
    if causal not in _NC_CACHE:
        _NC_CACHE[causal] = _build_nc(causal)
    return _NC_CACHE[causal]


def _host_consts():
    p = np.zeros((128, 128), np.float32)
    idx = np.arange(0, 128, 2)
    p[idx, idx + 1] = -1.0
    p[idx + 1, idx] = 1.0
    psigT = np.ascontiguousarray(p.T).astype(BF)
    pm = np.where(np.arange(128)[None, :] < np.arange(128)[:, None],
                  np.float32(NEG), np.float32(0.0))
    ptneg = np.ascontiguousarray(pm.T).astype(BF)
    ident = np.eye(128, dtype=np.float32).astype(BF)
    m384 = np.zeros((128, 384), np.float32)
    m384[:, 0:128] = np.eye(128)
    m384[:, 256:384] = np.eye(128)
    return psigT, ptneg, ident, m384.astype(BF)


def _numpy_reference(hidden_states, cos, sin, attention_mask, Wq, Wk, Wv, Wo):
    """Generic-mask fallback, pure numpy port of the reference."""
    GROUPS = H // KVH

    def rope(x, c, s):
        c = c[:, None, :, :]
        s = s[:, None, :, :]
        x1, x2 = x[..., ::2], x[..., 1::2]
        xr = np.stack([x1 * c - x2 * s, x1 * s + x2 * c], axis=-1)
        return xr.reshape(x.shape)

    b, sq, d = hidden_states.shape
    q = (hidden_states @ Wq).reshape(b, sq, H, HD).transpose(0, 2, 1, 3)
    k = (hidden_states @ Wk).reshape(b, sq, KVH, HD).transpose(0, 2, 1, 3)
    v = (hidden_states @ Wv).reshape(b, sq, KVH, HD).transpose(0, 2, 1, 3)
    q = rope(q, cos, sin)
    k = rope(k, cos, sin)
    k = np.repeat(k, GROUPS, axis=1)
    v = np.repeat(v, GROUPS, axis=1)
    out = np.zeros((b, sq, d), np.float32)
    for bi in range(b):
        for hi in range(H):
            sc = (q[bi, hi] @ k[bi, hi].T) * SCALE + attention_mask[0, 0]
            sc = sc - sc.max(axis=-1, keepdims=True)
            e = np.exp(sc)
            pr = e / e.sum(axis=-1, keepdims=True)
            ctx = pr @ v[bi, hi]
            out[bi] += ctx @ Wo[hi * HD:(hi + 1) * HD]
    return out


def kernel(**inputs) -> np.ndarray:
    hs = np.asarray(inputs["hidden_states"], np.float32)
    cos = np.asarray(inputs["cos"], np.float32)
    sin = np.asarray(inputs["sin"], np.float32)
    mask = np.asarray(inputs["attention_mask"], np.float32)
    Wq = np.asarray(inputs["Wq"], np.float32)
    Wk = np.asarray(inputs["Wk"], np.float32)
    Wv = np.asarray(inputs["Wv"], np.float32)
    Wo = np.asarray(inputs["Wo"], np.float32)

    m = mask.reshape(S, S)
    tril = np.tril(np.ones((S, S), dtype=bool))
    causal_ref = np.where(tril, np.float32(0.0), np.float32(NEG))
    if np.array_equal(m, causal_ref):
        causal = True
    elif not m.any():
        causal = False
    else:
        return _numpy_reference(hs, cos, sin, mask, Wq, Wk, Wv, Wo)

    nc = _get_nc(causal)
    psigT, ptneg, ident, m384 = _host_consts()
    chan_half = (np.arange(64) // 2)

    in_maps = []
    for core in range(8):
        b, t = core // TP, core % TP
        hTf = np.ascontiguousarray(hs[b].T)                       # [D, S]
        h1, h2 = _split8(hTf)
        ht8a = _pairs(h1, S)
        ht8b = _pairs(h2, S)
        cs64 = np.ascontiguousarray(cos[b].T[chan_half, :])       # [64, S]
        sn64 = np.ascontiguousarray(sin[b].T[chan_half, :])
        csd = (np.vstack([cs64, cs64]) / WS).astype(BF)
        snd = (np.vstack([sn64, sn64]) / WS).astype(BF)
        wq1, wq2 = _split8(Wq[:, t * 256:(t + 1) * 256] * WS)
        wkv_f = np.concatenate([Wk[:, t * 64:(t + 1) * 64],
                                Wv[:, t * 64:(t + 1) * 64]], axis=1)
        wk1, wk2 = _split8(wkv_f * WS)
        wo_s = np.ascontiguousarray(
            Wo[t * 256:(t + 1) * 256].reshape(2, 128, D)
            .transpose(1, 0, 2)).astype(BF)
        in_maps.append({
            "ht8a": ht8a, "ht8b": ht8b, "csd": csd, "snd": snd,
            "wq8a": _pairs(wq1, 256), "wq8b": _pairs(wq2, 256),
            "wkv8a": _pairs(wk1, 128), "wkv8b": _pairs(wk2, 128),
            "wo": wo_s,
            "psigT": psigT, "ptneg": ptneg, "ident": ident, "m384": m384,
        })

    res = run_bass_kernel_spmd(nc, in_maps, core_ids=list(range(8)))
    out = np.zeros((B, S, D), np.float32)
    for core in range(8):
        out[core // TP] += res.results[core]["out"].astype(np.float32)
    return out
